# revision 13
# baseline (speedup 1.0000x reference)
"""Trainium2 Bass kernel for causal multi-head attention.

Problem: B=2, S=2048, D=1024, H=16 heads (hd=64), fp32 in/out.
  qkv = x @ Wqkv + bqkv ; per-head causal softmax attention ; out = ctx @ Wo + bo

Sharding (8 NeuronCores): tensor-parallel over heads — 2 heads per core.
Each core computes q/k/v projections for its 2 heads (both batches), causal
attention, and its ctx^T slice [128 feat, B*S]. Four AllToAll collectives
(one per (batch, half), 256KB each) redistribute ctx^T from head-sharded to
row-sharded; each core then projects 128 rows per chunk with the full Wo.
Host reassembles the row slices.

v4 vs v2:
- All big inputs (x rounds, wqk, wv, wo) pre-tiled host-side into
  per-partition-contiguous DRAM layouts: DMA issue drops from ~1-8us to
  ~0.2us each and transfers run at full HBM rate; first matmul starts ~6us
  earlier.
- Ragged AV: the attn@v accumulation skips the causally-invalid q-ranges of
  the diagonal k-tiles (N=512-128*o) instead of multiplying zeroed exp.
  Saves ~25% of AV matmul columns and drops the exp-tile zero memsets.
- Fine-grained PE interleave: the AV matmuls of window j-1 are emitted
  between the scores pieces of window j (the PE is in-order, so filler must
  be interleaved in emission order). The PE no longer stalls while the
  Scalar engine drains exp.
- The per-batch AllToAll is split into four per-(batch, half) collectives,
  each fired as soon as its two windows' ctx is normalized; projections are
  spread between late windows so only a 256KB collective + one projection
  remain in the tail.

Numerics: bf16 matmul operands, fp32 PSUM accumulation. Softmax uses
exp without max-subtraction (scores are ~N(0,1) after the folded 1/sqrt(hd)
scale; |s| < ~8 so fp32 exp/sums are safe). The softmax denominator comes
for free as a ones-column appended to v in the attn@v matmul.
"""

import numpy as np
import ml_dtypes

B, S, D, H, NC = 2, 2048, 1024, 16, 8
HD = D // H            # 64
HPC = H // NC          # 2 heads per core
BS = B * S             # 4096
RPB = S // NC          # 256 output rows per core per batch
KC = D // 128          # 8 contraction chunks
NR = 4                 # qkv rounds of 1024 tokens
NKT = S // 128         # 16 k-tiles (128) per batch

BF16 = ml_dtypes.bfloat16

_CACHE = {}


def _build_program():
    import concourse.bass as bass
    import concourse.mybir as mybir
    from concourse import bacc
    from concourse.tile import TileContext

    dt = mybir.dt
    f32, bf16 = dt.float32, dt.bfloat16
    ALU = mybir.AluOpType
    ACTF = mybir.ActivationFunctionType

    nc = bacc.Bacc("TRN2", target_bir_lowering=False, debug=False, num_devices=NC)

    xr_d = [nc.dram_tensor(f"xr{r}", [128, KC, 1024], bf16,
                           kind="ExternalInput") for r in range(NR)]
    wqk = nc.dram_tensor("wqk", [128, KC, 256], bf16, kind="ExternalInput")
    wv = nc.dram_tensor("wv", [128, KC, 128], bf16, kind="ExternalInput")
    wo = nc.dram_tensor("wo", [128, KC, D], bf16, kind="ExternalInput")
    bqk = nc.dram_tensor("bqk", [128, 2], f32, kind="ExternalInput")
    bv = nc.dram_tensor("bv", [128, 1], f32, kind="ExternalInput")
    bo = nc.dram_tensor("bo", [128, D], f32, kind="ExternalInput")
    mask = nc.dram_tensor("mask", [128, 128], bf16, kind="ExternalInput")
    ident = nc.dram_tensor("ident", [128, 128], bf16, kind="ExternalInput")
    out = nc.dram_tensor("out", [2 * RPB, D], f32, kind="ExternalOutput")

    # collective buffers: one AllToAll per (batch, half). Shard j of the
    # send buffer = [our 128 feats, core j's 128 q rows of this half];
    # the received shard j = [core j's 128 feats, our 128 rows].
    ctx_dram = [[nc.dram_tensor(f"ctxb{g}h{h}", [NC, 128, 128], bf16)
                 for h in range(2)] for g in range(B)]
    a2a_dram = [[nc.dram_tensor(f"ctxa2a{g}h{h}", [NC, 128, 128], bf16)
                 for h in range(2)] for g in range(B)]

    with TileContext(nc) as tc:
        with (
            tc.tile_pool(name="const", bufs=1) as cpool,
            tc.tile_pool(name="big", bufs=1) as bigpool,
            tc.tile_pool(name="xstream", bufs=2) as xpool,
            tc.tile_pool(name="vt", bufs=2) as vtpool,
            tc.tile_pool(name="exp", bufs=1) as epool,
            tc.tile_pool(name="small", bufs=2) as spool,
            tc.tile_pool(name="ag", bufs=4) as agpool,
            tc.tile_pool(name="outp", bufs=2) as opool,
            tc.tile_pool(name="psA", bufs=2, space="PSUM") as psA,   # 2x [128,1024]
            tc.tile_pool(name="psB", bufs=4, space="PSUM") as psB,   # 4x [128,512]
        ):
            # ---- constants / weights to SBUF ----
            # wqk + round-0 x gate the whole kernel: queue them first, in
            # arrival-need order, all per-partition-contiguous in DRAM.
            wqk_sb = cpool.tile([128, KC, 256], bf16, tag="wqk")
            x0 = xpool.tile([128, KC, 1024], bf16, tag="xt", name="x0")
            x1 = xpool.tile([128, KC, 1024], bf16, tag="xt", name="x1")
            # per-queue DMA bandwidth saturates well below HBM rate, so the
            # x stream is split across the sync and scalar queues
            # fine-grained startup chunks round-robined over the three
            # DMA-capable queues: the first matmul needs only wqk kk0-1 +
            # x0 kk0, and the aggregate stream keeps round 0/1 fed densely
            qs = [nc.sync, nc.scalar, nc.gpsimd]
            nc.sync.dma_start(wqk_sb[:, 0:2, :], wqk[:, 0:2, :])
            nc.scalar.dma_start(wqk_sb[:, 2:4, :], wqk[:, 2:4, :])
            nc.gpsimd.dma_start(wqk_sb[:, 4:6, :], wqk[:, 4:6, :])
            nc.scalar.dma_start(wqk_sb[:, 6:8, :], wqk[:, 6:8, :])
            for kk in range(KC):
                qs[kk % 3].dma_start(x0[:, kk:kk + 1, :],
                                     xr_d[0][:, kk:kk + 1, :])
            for c in range(4):
                qs[c % 3].dma_start(x1[:, 2 * c:2 * c + 2, :],
                                    xr_d[1][:, 2 * c:2 * c + 2, :])
            wv_sb = cpool.tile([128, KC, 128], bf16, tag="wv")
            nc.scalar.dma_start(wv_sb[:], wv[:])
            bqk_sb = cpool.tile([128, 2], f32, tag="bqk")
            nc.scalar.dma_start(bqk_sb[:], bqk[:])
            bv_sb = cpool.tile([128, 1], f32, tag="bv")
            nc.scalar.dma_start(bv_sb[:], bv[:])
            mask_sb = cpool.tile([128, 128], bf16, tag="mask")
            nc.scalar.dma_start(mask_sb[:], mask[:])
            ident_sb = cpool.tile([128, 128], bf16, tag="ident")
            nc.scalar.dma_start(ident_sb[:], ident[:])
            wo_sb = cpool.tile([128, KC, D], bf16, tag="wo")
            bo_sb = cpool.tile([128, D], f32, tag="bo")

            # ---- persistent activations ----
            qT_sb = bigpool.tile([128, BS], bf16, tag="qT")   # [2*64 feat, B*S]
            kT_sb = bigpool.tile([128, BS], bf16, tag="kT")
            # v natural layout: [kpos, tile, head, 65] with ones at col 64
            v_sb = bigpool.tile([128, BS // 128, HPC, 65], bf16, tag="v")
            ctxT_sb = bigpool.tile([128, BS], bf16, tag="ctxT")

            nc.vector.memset(v_sb[:, :, :, 64:65], 1.0)

            vt_tiles = {}
            exp_tiles = {}

            # ---- qkv projection: rounds of 1024 tokens ----
            def emit_qkv_round(r):
                lo = r * 1024
                if r == 0:
                    xt = x0
                elif r == 1:
                    xt = x1
                else:
                    xt = xpool.tile([128, KC, 1024], bf16, tag="xt")
                    eng2 = nc.scalar if r == 2 else nc.gpsimd
                    for c in range(4):
                        eng = nc.sync if c % 2 == 0 else eng2
                        eng.dma_start(xt[:, 2 * c:2 * c + 2, :],
                                      xr_d[r][:, 2 * c:2 * c + 2, :])

                ps_q = psA.tile([128, 1024], f32, tag="psA", name="ps_q")
                ps_k = psA.tile([128, 1024], f32, tag="psA", name="ps_k")
                ps_v0 = psB.tile([128, 512], f32, tag="psB", name="ps_v0")
                ps_v1 = psB.tile([128, 512], f32, tag="psB", name="ps_v1")
                for kk in range(KC):
                    nc.tensor.matmul(ps_q[:, 0:512], lhsT=wqk_sb[:, kk, 0:128],
                                     rhs=xt[:, kk, 0:512],
                                     start=(kk == 0), stop=(kk == KC - 1))
                    nc.tensor.matmul(ps_q[:, 512:1024], lhsT=wqk_sb[:, kk, 0:128],
                                     rhs=xt[:, kk, 512:1024],
                                     start=(kk == 0), stop=(kk == KC - 1))
                for kk in range(KC):
                    nc.tensor.matmul(ps_k[:, 0:512], lhsT=wqk_sb[:, kk, 128:256],
                                     rhs=xt[:, kk, 0:512],
                                     start=(kk == 0), stop=(kk == KC - 1))
                    nc.tensor.matmul(ps_k[:, 512:1024], lhsT=wqk_sb[:, kk, 128:256],
                                     rhs=xt[:, kk, 512:1024],
                                     start=(kk == 0), stop=(kk == KC - 1))
                for kk in range(KC):
                    nc.tensor.matmul(ps_v0, lhsT=wv_sb[:, kk, :],
                                     rhs=xt[:, kk, 0:512],
                                     start=(kk == 0), stop=(kk == KC - 1))
                    nc.tensor.matmul(ps_v1, lhsT=wv_sb[:, kk, :],
                                     rhs=xt[:, kk, 512:1024],
                                     start=(kk == 0), stop=(kk == KC - 1))
                vt = vtpool.tile([128, 1024], bf16, tag="vT", name="vt")
                vt_tiles[r] = vt
                nc.vector.tensor_scalar_add(qT_sb[:, lo:lo + 1024], ps_q,
                                            bqk_sb[:, 0:1])
                nc.vector.tensor_scalar_add(kT_sb[:, lo:lo + 1024], ps_k,
                                            bqk_sb[:, 1:2])
                nc.vector.tensor_scalar_add(vt[:, 0:512], ps_v0,
                                            bv_sb[:, 0:1])
                nc.vector.tensor_scalar_add(vt[:, 512:1024], ps_v1,
                                            bv_sb[:, 0:1])

            # v^T [feat, tok] -> natural [tok, feat] via PE transposes,
            # 8 tiles packed per PSUM tile, drained by one strided DVE copy.
            def emit_v_transposes(r):
                vt = vt_tiles.pop(r)
                pack = psA.tile([128, 8, HPC, 64], bf16, tag="psA", name="tpack")
                for t8 in range(8):
                    c0 = t8 * 128
                    nc.tensor.transpose(pack[:, t8], vt[:, c0:c0 + 128],
                                        ident_sb[:])
                nc.vector.tensor_copy(v_sb[:, r * 8:(r + 1) * 8, :, 0:64],
                                      pack[:])

            # ---- collectives + output projection ----
            def emit_a2a(b, h):
                cols = slice(b * S + h * 1024, b * S + (h + 1) * 1024)
                nc.gpsimd.dma_start(
                    ctx_dram[b][h].rearrange("j p s -> p j s"),
                    ctxT_sb[:, cols])
                nc.gpsimd.collective_compute(
                    "AllToAll",
                    mybir.AluOpType.bypass,
                    replica_groups=[list(range(NC))],
                    ins=[ctx_dram[b][h][:]],
                    outs=[a2a_dram[b][h][:]],
                )

            def load_ctxag(g):
                b, half = g // 2, g % 2
                ctxag_sb = agpool.tile([128, NC, 128], bf16, tag="ctxag",
                                       name="ctxag_sb")
                src_v = a2a_dram[b][half].rearrange("j p s -> p j s")
                nc.sync.dma_start(ctxag_sb[:, 0:4, :], src_v[:, 0:4, :])
                nc.sync.dma_start(ctxag_sb[:, 4:8, :], src_v[:, 4:8, :])
                return ctxag_sb

            def emit_proj(g, ctxag_sb=None):
                if ctxag_sb is None:
                    ctxag_sb = load_ctxag(g)
                ps_o = psA.tile([128, 1024], f32, tag="psA", name="ps_o")
                # sequential halves: half-0's bias-add + store overlap
                # half-1's matmuls, shortening the tail after the last a2a
                for half2 in range(2):
                    cs = slice(512 * half2, 512 * half2 + 512)
                    for k in range(NC):
                        nc.tensor.matmul(ps_o[:, cs],
                                         lhsT=ctxag_sb[:, k, :],
                                         rhs=wo_sb[:, k, cs],
                                         start=(k == 0), stop=(k == NC - 1))
                    ot = opool.tile([128, 512], f32, tag="ot")
                    nc.vector.tensor_tensor(ot[:], ps_o[:, cs],
                                            bo_sb[:, cs], ALU.add)
                    nc.sync.dma_start(out[g * 128:(g + 1) * 128, cs], ot[:])

            # ---- AV + normalize, emitted as fine-grained filler ----
            # Returns a list of closures: PE matmul chunks (interleaved
            # between the next window's scores pieces to keep the in-order
            # PE busy) followed by one normalize closure per head (vector/
            # gpsimd work, order-free).
            def build_av_steps(b, j):
                steps = []
                nkt = 4 * (j + 1)
                for hl in range(HPC):
                    exp_j = exp_tiles[(j, hl)]
                    ps_c = psB.tile([128, 512], f32, tag="psB", name="ps_c")
                    # full k-tiles at N=512, then ragged diagonal tiles at
                    # N=512-128*o (the causally-invalid prefix is skipped)
                    mms = [(tt, 0) for tt in range(4 * j)]
                    mms += [(4 * j + o, 128 * o) for o in range(4)]

                    def mk_mm(pair, first, last, hl=hl, exp_j=exp_j,
                              ps_c=ps_c):
                        def go():
                            for i, (tt, q0) in enumerate(pair):
                                nc.tensor.matmul(
                                    ps_c[:65, q0:512],
                                    lhsT=v_sb[:, b * NKT + tt, hl, :],
                                    rhs=exp_j[:, tt * 512 + q0:
                                              (tt + 1) * 512],
                                    start=(first and i == 0),
                                    stop=(last and i == len(pair) - 1))
                        return go

                    for ci in range(0, len(mms), 2):
                        pair = mms[ci:ci + 2]
                        steps.append(mk_mm(pair, ci == 0,
                                           ci + 2 >= len(mms)))

                    def mk_norm(hl=hl, ps_c=ps_c):
                        def go():
                            hp = slice(64 * hl, 64 * hl + 64)
                            # den/recip read PSUM directly, in parallel with
                            # the stage copy — shortens the serial chain to
                            # the a2a trigger by ~3us
                            den = spool.tile([1, 512], f32, tag="den")
                            nc.vector.tensor_copy(den[:], ps_c[64:65, :])
                            recip = spool.tile([1, 512], f32, tag="recip")
                            nc.vector.reciprocal_approx_fast(out=recip[:],
                                                             in_=den[:])
                            stage = spool.tile([64, 512], f32, tag="stage",
                                               bufs=3)
                            nc.vector.tensor_copy(stage[:], ps_c[0:64, :])
                            bcast = spool.tile([64, 512], f32, tag="bcast",
                                               bufs=3)
                            nc.gpsimd.partition_broadcast(bcast[:], recip[:])
                            cs = slice(b * S + j * 512,
                                       b * S + (j + 1) * 512)
                            nc.vector.tensor_tensor(ctxT_sb[hp, cs],
                                                    stage[:], bcast[:],
                                                    ALU.mult)
                        return go

                    steps.append(mk_norm())
                return steps

            def emit_pe_warm(n):
                # dummy matmuls that keep the PE busy (and at max pstate)
                # while the final collective's peer-wait + transfer drain;
                # results are never read
                for i in range(n):
                    ps_w = psA.tile([128, 1024], f32, tag="psA",
                                    name="ps_warm")
                    nc.tensor.matmul(ps_w[:, 0:512],
                                     lhsT=wqk_sb[:, 0, 0:128],
                                     rhs=qT_sb[:, 0:512],
                                     start=True, stop=True)

            filler = []

            def run_filler(n):
                for _ in range(min(n, len(filler))):
                    filler.pop(0)()

            def drain_filler(before_last=None):
                while filler:
                    if before_last is not None and len(filler) == 1:
                        before_last()
                        before_last = None
                    filler.pop(0)()

            def emit_window(b, j):
                nkt = 4 * (j + 1)
                exp_js = []
                for hl in range(HPC):
                    t = epool.tile([128, nkt * 512], bf16,
                                   tag=f"expj{j}h{hl}", name="exp_j")
                    exp_tiles[(j, hl)] = t
                    exp_js.append(t)
                # Scores pieces: off-diagonal k-tiles (full 512-q) in pairs,
                # then the 4 diagonal tiles (ragged: tile 4j+o covers the
                # last 512-128*o q columns) packed into two PSUM tiles.
                win = b * S + j * 512
                pieces = []
                tt = 0
                while tt < 4 * j:
                    npc = min(2, 4 * j - tt)
                    pieces.append([(tt + i, 0) for i in range(npc)])
                    tt += npc
                pieces.append([(4 * j, 0), (4 * j + 1, 128)])
                pieces.append([(4 * j + 2, 256), (4 * j + 3, 384)])
                n_pieces = len(pieces)
                for pi, piece in enumerate(pieces):
                    ps_h = [psA.tile([128, 1024], f32, tag="psA",
                                     name="ps_sc")
                            for _ in range(HPC)]
                    # pack spans so no matmul output crosses a 512-col
                    # (2KB) PSUM bank boundary
                    col = 0
                    spans = []
                    for (tile_idx, qoff) in piece:
                        w = 512 - qoff
                        if col // 512 != (col + w - 1) // 512:
                            col = ((col + 511) // 512) * 512
                        spans.append((tile_idx, qoff, col, w))
                        col += w
                    for (tile_idx, qoff, c0, w) in spans:
                        kt = b * S + tile_idx * 128
                        for hl in range(HPC):
                            hp = slice(64 * hl, 64 * hl + 64)
                            nc.tensor.matmul(
                                ps_h[hl][:, c0:c0 + w],
                                lhsT=kT_sb[hp, kt:kt + 128],
                                rhs=qT_sb[hp, win + qoff:win + 512],
                                start=True, stop=True)
                    # exp: one ACT op per head per src/dst-contiguous run
                    for hl in range(HPC):
                        run = []
                        for (tile_idx, qoff, c0, w) in spans:
                            dst = tile_idx * 512 + qoff
                            if run and run[-1][1] + run[-1][2] == dst \
                                    and run[-1][0] + run[-1][2] == c0:
                                run[-1] = (run[-1][0], run[-1][1],
                                           run[-1][2] + w)
                            else:
                                run.append((c0, dst, w))
                        for (c0, dst, w) in run:
                            nc.scalar.activation(
                                exp_js[hl][:, dst:dst + w],
                                ps_h[hl][:, c0:c0 + w], ACTF.Exp)
                    # interleave AV filler of the previous window so the
                    # in-order PE has work while Scalar drains exp. Coarse
                    # grain (every 2nd piece): scores run in 64-row PE tile
                    # mode, AV in 128-row mode, and each mode switch drains
                    # the PE array
                    if filler and (pi % 2 == 1 or pi == n_pieces - 1):
                        rem = (n_pieces - pi + 1) // 2
                        per = -(-len(filler) // max(rem, 1))
                        run_filler(per)
                # triangular causal mask on each diagonal tile's first
                # 128 valid columns
                for hl in range(HPC):
                    for o in range(4):
                        lo = (4 * j + o) * 512 + 128 * o
                        nc.vector.tensor_tensor(exp_js[hl][:, lo:lo + 128],
                                                exp_js[hl][:, lo:lo + 128],
                                                mask_sb[:],
                                                ALU.mult)
                drain_filler()
                filler.extend(build_av_steps(b, j))

            scope1 = nc.named_scope("qkv"); scope1.__enter__()
            emit_qkv_round(0)
            emit_qkv_round(1)
            emit_v_transposes(0)
            emit_v_transposes(1)
            scope1.__exit__(None, None, None)

            scope2 = nc.named_scope("attn"); scope2.__enter__()
            emit_window(0, 0)
            emit_qkv_round(2)
            # wo/bo are needed only by the projections; queued on scalar so
            # rounds 2/3's x chunks never sit behind them
            nc.scalar.dma_start(wo_sb[:], wo[:])
            nc.scalar.dma_start(bo_sb[:], bo[:])
            emit_window(0, 1)
            emit_v_transposes(2)
            emit_qkv_round(3)
            emit_window(0, 2)
            emit_a2a(0, 0)             # data (norms of 0,0/0,1) ready
                                       # mid-window; store fires on data,
                                       # not emission position
            emit_v_transposes(3)
            emit_window(0, 3)
            emit_window(1, 0)
            emit_a2a(0, 1)
            emit_window(1, 1)
            ag0 = load_ctxag(0)
            emit_window(1, 2)
            emit_a2a(1, 0)
            ag1 = load_ctxag(1)
            emit_window(1, 3)
            drain_filler()             # AV(1,3) + norms
            # all four projections run in the tail, gated only by their
            # data: proj0-2's matmuls span the final collective's peer-wait
            # + transfer; the a2a(1,h1) store/trigger (emitted after
            # proj2 to keep the emission barrier off proj0-2) fires as soon
            # as the norms complete
            ag2 = load_ctxag(2)
            emit_proj(0, ag0)
            emit_proj(1, ag1)
            emit_proj(2, ag2)
            emit_a2a(1, 1)
            ag3 = load_ctxag(3)
            emit_proj(3, ag3)
            scope2.__exit__(None, None, None)

    nc.compile()
    return nc


def _prep_inputs(x, Wqkv, bqkv, Wo, bo):
    x = np.asarray(x, dtype=np.float32)
    Wqkv = np.asarray(Wqkv, dtype=np.float32)
    bqkv = np.asarray(bqkv, dtype=np.float32)
    Wo = np.asarray(Wo, dtype=np.float32)
    bo = np.asarray(bo, dtype=np.float32)

    xT = x.reshape(BS, D).T.astype(BF16)               # [D, BS]
    # per-round per-partition-contiguous tiles: xr[r][p, ko, t] =
    # xT[ko*128+p, r*1024+t]
    xr = np.ascontiguousarray(
        xT.reshape(KC, 128, NR, 1024).transpose(2, 1, 0, 3))
    wo_t = np.ascontiguousarray(
        Wo.astype(BF16).reshape(KC, 128, D).transpose(1, 0, 2))
    bo_t = np.tile(bo.astype(np.float32), (128, 1))

    kp = np.arange(128)[:, None]
    u = np.arange(128)[None, :]
    mask = (u >= kp).astype(BF16)
    ident = np.eye(128, dtype=BF16)

    scale = np.float32(1.0 / np.sqrt(HD))

    # Wqkv columns per head h: q = 192h..+64, k = +64, v = +128
    W3 = Wqkv.reshape(D, H, 3, HD)
    b3 = bqkv.reshape(H, 3, HD)

    in_maps = []
    for c in range(NC):
        hs = [HPC * c + i for i in range(HPC)]
        wq = np.concatenate([W3[:, h, 0, :] for h in hs], axis=1) * scale
        wk = np.concatenate([W3[:, h, 1, :] for h in hs], axis=1)
        wv_ = np.concatenate([W3[:, h, 2, :] for h in hs], axis=1)
        bq = np.concatenate([b3[h, 0, :] for h in hs]) * scale
        bk = np.concatenate([b3[h, 1, :] for h in hs])
        bv_ = np.concatenate([b3[h, 2, :] for h in hs])
        wqk_c = np.concatenate([wq, wk], axis=1).astype(BF16)   # [D, 256]
        wqk_t = np.ascontiguousarray(
            wqk_c.reshape(KC, 128, 256).transpose(1, 0, 2))
        wv_t = np.ascontiguousarray(
            wv_.astype(BF16).reshape(KC, 128, 128).transpose(1, 0, 2))
        m = {
            "wqk": wqk_t,
            "wv": wv_t,
            "wo": wo_t,
            "bqk": np.ascontiguousarray(
                np.stack([bq, bk], axis=1)).astype(np.float32),
            "bv": bv_.astype(np.float32).reshape(128, 1),
            "bo": bo_t,
            "mask": mask,
            "ident": ident,
        }
        for r in range(NR):
            m[f"xr{r}"] = xr[r]
        in_maps.append(m)
    return in_maps


def run(x, Wqkv, bqkv, Wo, bo, trace=False):
    from concourse.bass_utils import run_bass_kernel_spmd

    if "nc" not in _CACHE:
        _CACHE["nc"] = _build_program()
    nc = _CACHE["nc"]
    in_maps = _prep_inputs(x, Wqkv, bqkv, Wo, bo)
    res = run_bass_kernel_spmd(nc, in_maps, list(range(NC)), trace=trace)
    # core c returns [512, D]: 4 chunks of 128 rows: (b0 rows 128c..),
    # (b0 rows 1024+128c..), (b1 rows 128c..), (b1 rows 1024+128c..)
    full = np.empty((B, S, D), dtype=np.float32)
    for c in range(NC):
        r = res.results[c]["out"]
        for g in range(4):
            b, half = g // 2, g % 2
            lo = half * 1024 + 128 * c
            full[b, lo:lo + 128, :] = r[g * 128:(g + 1) * 128, :]
    return full, res


def kernel(x, Wqkv, bqkv, Wo, bo):
    out, _ = run(x, Wqkv, bqkv, Wo, bo)
    return out


# revision 16
# speedup vs baseline: 1.1779x; 1.1779x over previous
"""Trainium2 Bass kernel for causal multi-head attention.

Problem: B=2, S=2048, D=1024, H=16 heads (hd=64), fp32 in/out.
  qkv = x @ Wqkv + bqkv ; per-head causal softmax attention ; out = ctx @ Wo + bo

Sharding (8 NeuronCores): tensor-parallel over heads — 2 heads per core.
Each core computes q/k/v projections for its 2 heads (both batches), causal
attention, and its ctx^T slice [128 feat, B*S]. Four AllToAll collectives
(one per (batch, half), 256KB each) redistribute ctx^T from head-sharded to
row-sharded; each core then projects 128 rows per chunk with the full Wo.
Host reassembles the row slices.

v4 vs v2:
- All big inputs (x rounds, wqk, wv, wo) pre-tiled host-side into
  per-partition-contiguous DRAM layouts: DMA issue drops from ~1-8us to
  ~0.2us each and transfers run at full HBM rate; first matmul starts ~6us
  earlier.
- Ragged AV: the attn@v accumulation skips the causally-invalid q-ranges of
  the diagonal k-tiles (N=512-128*o) instead of multiplying zeroed exp.
  Saves ~25% of AV matmul columns and drops the exp-tile zero memsets.
- Fine-grained PE interleave: the AV matmuls of window j-1 are emitted
  between the scores pieces of window j (the PE is in-order, so filler must
  be interleaved in emission order). The PE no longer stalls while the
  Scalar engine drains exp.
- The per-batch AllToAll is split into four per-(batch, half) collectives,
  each fired as soon as its two windows' ctx is normalized; projections are
  spread between late windows so only a 256KB collective + one projection
  remain in the tail.

Numerics: bf16 matmul operands, fp32 PSUM accumulation. Softmax uses
exp without max-subtraction (scores are ~N(0,1) after the folded 1/sqrt(hd)
scale; |s| < ~8 so fp32 exp/sums are safe). The softmax denominator comes
for free as a ones-column appended to v in the attn@v matmul.
"""

import numpy as np
import ml_dtypes

B, S, D, H, NC = 2, 2048, 1024, 16, 8
HD = D // H            # 64
HPC = H // NC          # 2 heads per core
BS = B * S             # 4096
RPB = S // NC          # 256 output rows per core per batch
KC = D // 128          # 8 contraction chunks
NR = 4                 # qkv rounds of 1024 tokens
NKT = S // 128         # 16 k-tiles (128) per batch

BF16 = ml_dtypes.bfloat16

_CACHE = {}


def _build_program():
    import concourse.bass as bass
    import concourse.mybir as mybir
    from concourse import bacc
    from concourse.tile import TileContext

    dt = mybir.dt
    f32, bf16 = dt.float32, dt.bfloat16
    ALU = mybir.AluOpType
    ACTF = mybir.ActivationFunctionType

    nc = bacc.Bacc("TRN2", target_bir_lowering=False, debug=False, num_devices=NC)

    xr_d = [nc.dram_tensor(f"xr{r}", [128, KC, 1024], bf16,
                           kind="ExternalInput") for r in range(NR)]
    wqk = nc.dram_tensor("wqk", [128, KC, 256], bf16, kind="ExternalInput")
    wv = nc.dram_tensor("wv", [128, KC, 128], bf16, kind="ExternalInput")
    wo = nc.dram_tensor("wo", [128, KC, D], bf16, kind="ExternalInput")
    bqk = nc.dram_tensor("bqk", [128, 2], f32, kind="ExternalInput")
    bv = nc.dram_tensor("bv", [128, 1], f32, kind="ExternalInput")
    bo = nc.dram_tensor("bo", [128, D], bf16, kind="ExternalInput")
    mask = nc.dram_tensor("mask", [128, 128], bf16, kind="ExternalInput")
    ident = nc.dram_tensor("ident", [128, 128], bf16, kind="ExternalInput")
    out = nc.dram_tensor("out", [2 * RPB, D], f32, kind="ExternalOutput")

    # collective buffers: one AllToAll per (batch, half). Shard j of the
    # send buffer = [our 128 feats, core j's 128 q rows of this half];
    # the received shard j = [core j's 128 feats, our 128 rows].
    ctx_dram = [[nc.dram_tensor(f"ctxb{g}h{h}", [NC, 128, 128], bf16)
                 for h in range(2)] for g in range(B)]
    a2a_dram = [[nc.dram_tensor(f"ctxa2a{g}h{h}", [NC, 128, 128], bf16)
                 for h in range(2)] for g in range(B)]

    with TileContext(nc) as tc:
        with (
            tc.tile_pool(name="const", bufs=1) as cpool,
            tc.tile_pool(name="big", bufs=1) as bigpool,
            tc.tile_pool(name="xstream", bufs=2) as xpool,
            tc.tile_pool(name="vt", bufs=2) as vtpool,
            tc.tile_pool(name="exp", bufs=1) as epool,
            tc.tile_pool(name="small", bufs=2) as spool,
            tc.tile_pool(name="ag", bufs=3) as agpool,
            tc.tile_pool(name="outp", bufs=2) as opool,
            tc.tile_pool(name="psA", bufs=2, space="PSUM") as psA,   # 2x [128,1024]
            tc.tile_pool(name="psB", bufs=4, space="PSUM") as psB,   # 4x [128,512]
        ):
            # ---- constants / weights to SBUF ----
            # wqk + round-0 x gate the whole kernel: queue them first, in
            # arrival-need order, all per-partition-contiguous in DRAM.
            wqk_sb = cpool.tile([128, KC, 256], bf16, tag="wqk")
            x0 = xpool.tile([128, KC, 1024], bf16, tag="xt", name="x0")
            x1 = xpool.tile([128, KC, 1024], bf16, tag="xt", name="x1")
            # per-queue DMA bandwidth saturates well below HBM rate, so the
            # x stream is split across the sync and scalar queues
            # fine-grained startup chunks round-robined over the three
            # DMA-capable queues: the first matmul needs only wqk kk0-1 +
            # x0 kk0, and the aggregate stream keeps round 0/1 fed densely
            qs = [nc.sync, nc.scalar, nc.gpsimd]
            nc.sync.dma_start(wqk_sb[:, 0:2, :], wqk[:, 0:2, :])
            nc.scalar.dma_start(wqk_sb[:, 2:4, :], wqk[:, 2:4, :])
            nc.gpsimd.dma_start(wqk_sb[:, 4:6, :], wqk[:, 4:6, :])
            nc.scalar.dma_start(wqk_sb[:, 6:8, :], wqk[:, 6:8, :])
            for kk in range(KC):
                qs[kk % 3].dma_start(x0[:, kk:kk + 1, :],
                                     xr_d[0][:, kk:kk + 1, :])
            for c in range(4):
                qs[c % 3].dma_start(x1[:, 2 * c:2 * c + 2, :],
                                    xr_d[1][:, 2 * c:2 * c + 2, :])
            wv_sb = cpool.tile([128, KC, 128], bf16, tag="wv")
            nc.scalar.dma_start(wv_sb[:], wv[:])
            bqk_sb = cpool.tile([128, 2], f32, tag="bqk")
            nc.scalar.dma_start(bqk_sb[:], bqk[:])
            bv_sb = cpool.tile([128, 1], f32, tag="bv")
            nc.scalar.dma_start(bv_sb[:], bv[:])
            mask_sb = cpool.tile([128, 128], bf16, tag="mask")
            nc.scalar.dma_start(mask_sb[:], mask[:])
            ident_sb = cpool.tile([128, 128], bf16, tag="ident")
            nc.scalar.dma_start(ident_sb[:], ident[:])
            wo_sb = cpool.tile([128, KC, D], bf16, tag="wo")
            bo_sb = cpool.tile([128, D], bf16, tag="bo")

            # ---- persistent activations ----
            # q stored twice, one copy per head with the other head's rows
            # zeroed: scores matmuls contract K=128 so the PE stays in
            # 128-row tile mode (64-row mode entries drain the array)
            qTp = [bigpool.tile([128, BS], bf16, tag=f"qT{hl}",
                                name=f"qT{hl}")
                   for hl in range(HPC)]
            kT_sb = bigpool.tile([128, BS], bf16, tag="kT")
            # v natural layout: [kpos, tile, head, 65] with ones at col 64
            v_sb = bigpool.tile([128, BS // 128, HPC, 65], bf16, tag="v")
            ctxT_sb = bigpool.tile([128, BS], bf16, tag="ctxT")

            nc.vector.memset(v_sb[:, :, :, 64:65], 1.0)
            nc.vector.memset(qTp[0][64:128, :], 0.0)
            nc.vector.memset(qTp[1][0:64, :], 0.0)

            vt_tiles = {}
            exp_tiles = {}

            # ---- qkv projection: rounds of 1024 tokens ----
            def emit_qkv_round(r):
                lo = r * 1024
                if r == 0:
                    xt = x0
                elif r == 1:
                    xt = x1
                else:
                    xt = xpool.tile([128, KC, 1024], bf16, tag="xt")
                    eng2 = nc.scalar if r == 2 else nc.gpsimd
                    for c in range(4):
                        eng = nc.sync if c % 2 == 0 else eng2
                        eng.dma_start(xt[:, 2 * c:2 * c + 2, :],
                                      xr_d[r][:, 2 * c:2 * c + 2, :])

                ps_q = psA.tile([128, 1024], f32, tag="psA", name="ps_q")
                ps_k = psA.tile([128, 1024], f32, tag="psA", name="ps_k")
                ps_v0 = psB.tile([128, 512], f32, tag="psB", name="ps_v0")
                ps_v1 = psB.tile([128, 512], f32, tag="psB", name="ps_v1")
                for kk in range(KC):
                    nc.tensor.matmul(ps_q[:, 0:512], lhsT=wqk_sb[:, kk, 0:128],
                                     rhs=xt[:, kk, 0:512],
                                     start=(kk == 0), stop=(kk == KC - 1))
                    nc.tensor.matmul(ps_q[:, 512:1024], lhsT=wqk_sb[:, kk, 0:128],
                                     rhs=xt[:, kk, 512:1024],
                                     start=(kk == 0), stop=(kk == KC - 1))
                for kk in range(KC):
                    nc.tensor.matmul(ps_k[:, 0:512], lhsT=wqk_sb[:, kk, 128:256],
                                     rhs=xt[:, kk, 0:512],
                                     start=(kk == 0), stop=(kk == KC - 1))
                    nc.tensor.matmul(ps_k[:, 512:1024], lhsT=wqk_sb[:, kk, 128:256],
                                     rhs=xt[:, kk, 512:1024],
                                     start=(kk == 0), stop=(kk == KC - 1))
                for kk in range(KC):
                    nc.tensor.matmul(ps_v0, lhsT=wv_sb[:, kk, :],
                                     rhs=xt[:, kk, 0:512],
                                     start=(kk == 0), stop=(kk == KC - 1))
                    nc.tensor.matmul(ps_v1, lhsT=wv_sb[:, kk, :],
                                     rhs=xt[:, kk, 512:1024],
                                     start=(kk == 0), stop=(kk == KC - 1))
                vt = vtpool.tile([128, 1024], bf16, tag="vT", name="vt")
                vt_tiles[r] = vt
                nc.vector.tensor_scalar_add(qTp[0][0:64, lo:lo + 1024],
                                            ps_q[0:64, :], bqk_sb[0:64, 0:1])
                nc.vector.tensor_scalar_add(qTp[1][64:128, lo:lo + 1024],
                                            ps_q[64:128, :],
                                            bqk_sb[64:128, 0:1])
                nc.vector.tensor_scalar_add(kT_sb[:, lo:lo + 1024], ps_k,
                                            bqk_sb[:, 1:2])
                nc.vector.tensor_scalar_add(vt[:, 0:512], ps_v0,
                                            bv_sb[:, 0:1])
                nc.vector.tensor_scalar_add(vt[:, 512:1024], ps_v1,
                                            bv_sb[:, 0:1])

            # v^T [feat, tok] -> natural [tok, feat] via PE transposes,
            # 8 tiles packed per PSUM tile, drained by one strided DVE copy.
            def emit_v_transposes(r):
                vt = vt_tiles.pop(r)
                pack = psA.tile([128, 8, HPC, 64], bf16, tag="psA", name="tpack")
                for t8 in range(8):
                    c0 = t8 * 128
                    nc.tensor.transpose(pack[:, t8], vt[:, c0:c0 + 128],
                                        ident_sb[:])
                nc.vector.tensor_copy(v_sb[:, r * 8:(r + 1) * 8, :, 0:64],
                                      pack[:])

            # ---- collectives + output projection ----
            def emit_a2a(b, h):
                cols = slice(b * S + h * 1024, b * S + (h + 1) * 1024)
                nc.gpsimd.dma_start(
                    ctx_dram[b][h].rearrange("j p s -> p j s"),
                    ctxT_sb[:, cols])
                nc.gpsimd.collective_compute(
                    "AllToAll",
                    mybir.AluOpType.bypass,
                    replica_groups=[list(range(NC))],
                    ins=[ctx_dram[b][h][:]],
                    outs=[a2a_dram[b][h][:]],
                )

            def load_ctxag(g):
                b, half = g // 2, g % 2
                ctxag_sb = agpool.tile([128, NC, 128], bf16, tag="ctxag",
                                       name="ctxag_sb")
                src_v = a2a_dram[b][half].rearrange("j p s -> p j s")
                nc.sync.dma_start(ctxag_sb[:, 0:4, :], src_v[:, 0:4, :])
                nc.sync.dma_start(ctxag_sb[:, 4:8, :], src_v[:, 4:8, :])
                return ctxag_sb

            def emit_proj(g, ctxag_sb=None):
                if ctxag_sb is None:
                    ctxag_sb = load_ctxag(g)
                ps_o = psA.tile([128, 1024], f32, tag="psA", name="ps_o")
                # sequential halves: half-0's bias-add + store overlap
                # half-1's matmuls, shortening the tail after the last a2a
                for half2 in range(2):
                    cs = slice(512 * half2, 512 * half2 + 512)
                    for k in range(NC):
                        nc.tensor.matmul(ps_o[:, cs],
                                         lhsT=ctxag_sb[:, k, :],
                                         rhs=wo_sb[:, k, cs],
                                         start=(k == 0), stop=(k == NC - 1))
                    ot = opool.tile([128, 512], f32, tag="ot")
                    nc.vector.tensor_tensor(ot[:], ps_o[:, cs],
                                            bo_sb[:, cs], ALU.add)
                    nc.sync.dma_start(out[g * 128:(g + 1) * 128, cs], ot[:])

            # ---- AV + normalize, emitted as fine-grained filler ----
            # Returns a list of closures: PE matmul chunks (interleaved
            # between the next window's scores pieces to keep the in-order
            # PE busy) followed by one normalize closure per head (vector/
            # gpsimd work, order-free).
            def build_av_steps(b, j):
                steps = []
                nkt = 4 * (j + 1)
                for hl in range(HPC):
                    exp_j = exp_tiles[(j, hl)]
                    ps_c = psB.tile([128, 512], f32, tag="psB", name="ps_c")
                    # full k-tiles at N=512, then ragged diagonal tiles at
                    # N=512-128*o (the causally-invalid prefix is skipped)
                    mms = [(tt, 0) for tt in range(4 * j)]
                    mms += [(4 * j + o, 128 * o) for o in range(4)]

                    def mk_mm(pair, first, last, hl=hl, exp_j=exp_j,
                              ps_c=ps_c):
                        def go():
                            for i, (tt, q0) in enumerate(pair):
                                nc.tensor.matmul(
                                    ps_c[:65, q0:512],
                                    lhsT=v_sb[:, b * NKT + tt, hl, :],
                                    rhs=exp_j[:, tt * 512 + q0:
                                              (tt + 1) * 512],
                                    start=(first and i == 0),
                                    stop=(last and i == len(pair) - 1))
                        return go

                    for ci in range(0, len(mms), 2):
                        pair = mms[ci:ci + 2]
                        steps.append(mk_mm(pair, ci == 0,
                                           ci + 2 >= len(mms)))

                    def mk_norm(hl=hl, ps_c=ps_c):
                        def go():
                            hp = slice(64 * hl, 64 * hl + 64)
                            # den/recip read PSUM directly, in parallel with
                            # the stage copy — shortens the serial chain to
                            # the a2a trigger by ~3us
                            den = spool.tile([1, 512], f32, tag="den")
                            nc.vector.tensor_copy(den[:], ps_c[64:65, :])
                            recip = spool.tile([1, 512], f32, tag="recip")
                            nc.vector.reciprocal_approx_fast(out=recip[:],
                                                             in_=den[:])
                            stage = spool.tile([64, 512], f32, tag="stage",
                                               bufs=2)
                            nc.vector.tensor_copy(stage[:], ps_c[0:64, :])
                            bcast = spool.tile([64, 512], f32, tag="bcast",
                                               bufs=2)
                            nc.gpsimd.partition_broadcast(bcast[:], recip[:])
                            cs = slice(b * S + j * 512,
                                       b * S + (j + 1) * 512)
                            nc.vector.tensor_tensor(ctxT_sb[hp, cs],
                                                    stage[:], bcast[:],
                                                    ALU.mult)
                        return go

                    steps.append(mk_norm())
                return steps

            def emit_pe_warm(n):
                # dummy matmuls that keep the PE busy (and at max pstate)
                # while the final collective's peer-wait + transfer drain;
                # results are never read
                for i in range(n):
                    ps_w = psA.tile([128, 1024], f32, tag="psA",
                                    name="ps_warm")
                    nc.tensor.matmul(ps_w[:, 0:512],
                                     lhsT=wqk_sb[:, 0, 0:128],
                                     rhs=qT_sb[:, 0:512],
                                     start=True, stop=True)

            filler = []

            def run_filler(n):
                for _ in range(min(n, len(filler))):
                    filler.pop(0)()

            def drain_filler(before_last=None):
                while filler:
                    if before_last is not None and len(filler) == 1:
                        before_last()
                        before_last = None
                    filler.pop(0)()

            def emit_window(b, j):
                nkt = 4 * (j + 1)
                exp_js = []
                for hl in range(HPC):
                    t = epool.tile([128, nkt * 512], bf16,
                                   tag=f"expj{j}h{hl}", name="exp_j")
                    exp_tiles[(j, hl)] = t
                    exp_js.append(t)
                # Scores pieces: off-diagonal k-tiles (full 512-q) in pairs,
                # then the 4 diagonal tiles (ragged: tile 4j+o covers the
                # last 512-128*o q columns) packed into two PSUM tiles.
                win = b * S + j * 512
                pieces = []
                tt = 0
                while tt < 4 * j:
                    npc = min(2, 4 * j - tt)
                    pieces.append([(tt + i, 0) for i in range(npc)])
                    tt += npc
                pieces.append([(4 * j, 0), (4 * j + 1, 128)])
                pieces.append([(4 * j + 2, 256), (4 * j + 3, 384)])
                n_pieces = len(pieces)
                for pi, piece in enumerate(pieces):
                    ps_h = [psA.tile([128, 1024], f32, tag="psA",
                                     name="ps_sc")
                            for _ in range(HPC)]
                    # pack spans so no matmul output crosses a 512-col
                    # (2KB) PSUM bank boundary
                    col = 0
                    spans = []
                    for (tile_idx, qoff) in piece:
                        w = 512 - qoff
                        if col // 512 != (col + w - 1) // 512:
                            col = ((col + 511) // 512) * 512
                        spans.append((tile_idx, qoff, col, w))
                        col += w
                    for (tile_idx, qoff, c0, w) in spans:
                        kt = b * S + tile_idx * 128
                        for hl in range(HPC):
                            nc.tensor.matmul(
                                ps_h[hl][:, c0:c0 + w],
                                lhsT=kT_sb[:, kt:kt + 128],
                                rhs=qTp[hl][:, win + qoff:win + 512],
                                start=True, stop=True)
                    # exp: one ACT op per head per src/dst-contiguous run
                    for hl in range(HPC):
                        run = []
                        for (tile_idx, qoff, c0, w) in spans:
                            dst = tile_idx * 512 + qoff
                            if run and run[-1][1] + run[-1][2] == dst \
                                    and run[-1][0] + run[-1][2] == c0:
                                run[-1] = (run[-1][0], run[-1][1],
                                           run[-1][2] + w)
                            else:
                                run.append((c0, dst, w))
                        for (c0, dst, w) in run:
                            nc.scalar.activation(
                                exp_js[hl][:, dst:dst + w],
                                ps_h[hl][:, c0:c0 + w], ACTF.Exp)
                    # interleave AV filler of the previous window so the
                    # in-order PE has work while Scalar drains exp. Coarse
                    # grain (every 2nd piece): scores run in 64-row PE tile
                    # mode, AV in 128-row mode, and each mode switch drains
                    # the PE array
                    if filler and (pi % 2 == 1 or pi == n_pieces - 1):
                        rem = (n_pieces - pi + 1) // 2
                        per = -(-len(filler) // max(rem, 1))
                        run_filler(per)
                # triangular causal mask on each diagonal tile's first
                # 128 valid columns
                for hl in range(HPC):
                    for o in range(4):
                        lo = (4 * j + o) * 512 + 128 * o
                        nc.vector.tensor_tensor(exp_js[hl][:, lo:lo + 128],
                                                exp_js[hl][:, lo:lo + 128],
                                                mask_sb[:],
                                                ALU.mult)
                drain_filler()
                filler.extend(build_av_steps(b, j))

            scope1 = nc.named_scope("qkv"); scope1.__enter__()
            emit_qkv_round(0)
            emit_qkv_round(1)
            emit_v_transposes(0)
            emit_v_transposes(1)
            scope1.__exit__(None, None, None)

            scope2 = nc.named_scope("attn"); scope2.__enter__()
            emit_window(0, 0)
            emit_qkv_round(2)
            # wo/bo are needed only by the projections; queued on scalar so
            # rounds 2/3's x chunks never sit behind them
            nc.scalar.dma_start(wo_sb[:], wo[:])
            nc.scalar.dma_start(bo_sb[:], bo[:])
            emit_window(0, 1)
            emit_v_transposes(2)
            emit_qkv_round(3)
            emit_window(0, 2)
            emit_a2a(0, 0)             # data (norms of 0,0/0,1) ready
                                       # mid-window; store fires on data,
                                       # not emission position
            emit_v_transposes(3)
            emit_window(0, 3)
            emit_window(1, 0)
            emit_a2a(0, 1)
            emit_window(1, 1)
            ag0 = load_ctxag(0)
            emit_window(1, 2)
            emit_a2a(1, 0)
            ag1 = load_ctxag(1)
            emit_window(1, 3)
            drain_filler()             # AV(1,3) + norms
            # all four projections run in the tail, gated only by their
            # data: proj0-2's matmuls span the final collective's peer-wait
            # + transfer; the a2a(1,h1) store/trigger (emitted after
            # proj2 to keep the emission barrier off proj0-2) fires as soon
            # as the norms complete
            ag2 = load_ctxag(2)
            emit_proj(0, ag0)
            emit_proj(1, ag1)
            emit_proj(2, ag2)
            emit_a2a(1, 1)
            ag3 = load_ctxag(3)
            emit_proj(3, ag3)
            scope2.__exit__(None, None, None)

    nc.compile()
    return nc


def _prep_inputs(x, Wqkv, bqkv, Wo, bo):
    x = np.asarray(x, dtype=np.float32)
    Wqkv = np.asarray(Wqkv, dtype=np.float32)
    bqkv = np.asarray(bqkv, dtype=np.float32)
    Wo = np.asarray(Wo, dtype=np.float32)
    bo = np.asarray(bo, dtype=np.float32)

    xT = x.reshape(BS, D).T.astype(BF16)               # [D, BS]
    # per-round per-partition-contiguous tiles: xr[r][p, ko, t] =
    # xT[ko*128+p, r*1024+t]
    xr = np.ascontiguousarray(
        xT.reshape(KC, 128, NR, 1024).transpose(2, 1, 0, 3))
    wo_t = np.ascontiguousarray(
        Wo.astype(BF16).reshape(KC, 128, D).transpose(1, 0, 2))
    bo_t = np.tile(bo.astype(BF16), (128, 1))

    kp = np.arange(128)[:, None]
    u = np.arange(128)[None, :]
    mask = (u >= kp).astype(BF16)
    ident = np.eye(128, dtype=BF16)

    scale = np.float32(1.0 / np.sqrt(HD))

    # Wqkv columns per head h: q = 192h..+64, k = +64, v = +128
    W3 = Wqkv.reshape(D, H, 3, HD)
    b3 = bqkv.reshape(H, 3, HD)

    in_maps = []
    for c in range(NC):
        hs = [HPC * c + i for i in range(HPC)]
        wq = np.concatenate([W3[:, h, 0, :] for h in hs], axis=1) * scale
        wk = np.concatenate([W3[:, h, 1, :] for h in hs], axis=1)
        wv_ = np.concatenate([W3[:, h, 2, :] for h in hs], axis=1)
        bq = np.concatenate([b3[h, 0, :] for h in hs]) * scale
        bk = np.concatenate([b3[h, 1, :] for h in hs])
        bv_ = np.concatenate([b3[h, 2, :] for h in hs])
        wqk_c = np.concatenate([wq, wk], axis=1).astype(BF16)   # [D, 256]
        wqk_t = np.ascontiguousarray(
            wqk_c.reshape(KC, 128, 256).transpose(1, 0, 2))
        wv_t = np.ascontiguousarray(
            wv_.astype(BF16).reshape(KC, 128, 128).transpose(1, 0, 2))
        m = {
            "wqk": wqk_t,
            "wv": wv_t,
            "wo": wo_t,
            "bqk": np.ascontiguousarray(
                np.stack([bq, bk], axis=1)).astype(np.float32),
            "bv": bv_.astype(np.float32).reshape(128, 1),
            "bo": bo_t,
            "mask": mask,
            "ident": ident,
        }
        for r in range(NR):
            m[f"xr{r}"] = xr[r]
        in_maps.append(m)
    return in_maps


def run(x, Wqkv, bqkv, Wo, bo, trace=False):
    from concourse.bass_utils import run_bass_kernel_spmd

    if "nc" not in _CACHE:
        _CACHE["nc"] = _build_program()
    nc = _CACHE["nc"]
    in_maps = _prep_inputs(x, Wqkv, bqkv, Wo, bo)
    res = run_bass_kernel_spmd(nc, in_maps, list(range(NC)), trace=trace)
    # core c returns [512, D]: 4 chunks of 128 rows: (b0 rows 128c..),
    # (b0 rows 1024+128c..), (b1 rows 128c..), (b1 rows 1024+128c..)
    full = np.empty((B, S, D), dtype=np.float32)
    for c in range(NC):
        r = res.results[c]["out"]
        for g in range(4):
            b, half = g // 2, g % 2
            lo = half * 1024 + 128 * c
            full[b, lo:lo + 128, :] = r[g * 128:(g + 1) * 128, :]
    return full, res


def kernel(x, Wqkv, bqkv, Wo, bo):
    out, _ = run(x, Wqkv, bqkv, Wo, bo)
    return out


# revision 17
# speedup vs baseline: 1.1825x; 1.0038x over previous
"""Trainium2 Bass kernel for causal multi-head attention.

Problem: B=2, S=2048, D=1024, H=16 heads (hd=64), fp32 in/out.
  qkv = x @ Wqkv + bqkv ; per-head causal softmax attention ; out = ctx @ Wo + bo

Sharding (8 NeuronCores): tensor-parallel over heads — 2 heads per core.
Each core computes q/k/v projections for its 2 heads (both batches), causal
attention, and its ctx^T slice [128 feat, B*S]. Four AllToAll collectives
(one per (batch, half), 256KB each) redistribute ctx^T from head-sharded to
row-sharded; each core then projects 128 rows per chunk with the full Wo.
Host reassembles the row slices.

v4 vs v2:
- All big inputs (x rounds, wqk, wv, wo) pre-tiled host-side into
  per-partition-contiguous DRAM layouts: DMA issue drops from ~1-8us to
  ~0.2us each and transfers run at full HBM rate; first matmul starts ~6us
  earlier.
- Ragged AV: the attn@v accumulation skips the causally-invalid q-ranges of
  the diagonal k-tiles (N=512-128*o) instead of multiplying zeroed exp.
  Saves ~25% of AV matmul columns and drops the exp-tile zero memsets.
- Fine-grained PE interleave: the AV matmuls of window j-1 are emitted
  between the scores pieces of window j (the PE is in-order, so filler must
  be interleaved in emission order). The PE no longer stalls while the
  Scalar engine drains exp.
- The per-batch AllToAll is split into four per-(batch, half) collectives,
  each fired as soon as its two windows' ctx is normalized; projections are
  spread between late windows so only a 256KB collective + one projection
  remain in the tail.

Numerics: bf16 matmul operands, fp32 PSUM accumulation. Softmax uses
exp without max-subtraction (scores are ~N(0,1) after the folded 1/sqrt(hd)
scale; |s| < ~8 so fp32 exp/sums are safe). The softmax denominator comes
for free as a ones-column appended to v in the attn@v matmul.
"""

import numpy as np
import ml_dtypes

B, S, D, H, NC = 2, 2048, 1024, 16, 8
HD = D // H            # 64
HPC = H // NC          # 2 heads per core
BS = B * S             # 4096
RPB = S // NC          # 256 output rows per core per batch
KC = D // 128          # 8 contraction chunks
NR = 4                 # qkv rounds of 1024 tokens
NKT = S // 128         # 16 k-tiles (128) per batch

BF16 = ml_dtypes.bfloat16

_CACHE = {}


def _build_program():
    import concourse.bass as bass
    import concourse.mybir as mybir
    from concourse import bacc
    from concourse.tile import TileContext

    dt = mybir.dt
    f32, bf16 = dt.float32, dt.bfloat16
    ALU = mybir.AluOpType
    ACTF = mybir.ActivationFunctionType

    nc = bacc.Bacc("TRN2", target_bir_lowering=False, debug=False, num_devices=NC)

    xr_d = [nc.dram_tensor(f"xr{r}", [128, KC, 1024], bf16,
                           kind="ExternalInput") for r in range(NR)]
    wqk = nc.dram_tensor("wqk", [128, KC, 256], bf16, kind="ExternalInput")
    wv = nc.dram_tensor("wv", [128, KC, 128], bf16, kind="ExternalInput")
    wo = nc.dram_tensor("wo", [128, KC, D], bf16, kind="ExternalInput")
    bqk = nc.dram_tensor("bqk", [128, 2], f32, kind="ExternalInput")
    bv = nc.dram_tensor("bv", [128, 1], f32, kind="ExternalInput")
    bo = nc.dram_tensor("bo", [128, D], bf16, kind="ExternalInput")
    mask = nc.dram_tensor("mask", [128, 128], bf16, kind="ExternalInput")
    ident = nc.dram_tensor("ident", [128, 128], bf16, kind="ExternalInput")
    out = nc.dram_tensor("out", [2 * RPB, D], f32, kind="ExternalOutput")

    # collective buffers: one AllToAll per (batch, half). Shard j of the
    # send buffer = [our 128 feats, core j's 128 q rows of this half];
    # the received shard j = [core j's 128 feats, our 128 rows].
    ctx_dram = [[nc.dram_tensor(f"ctxb{g}h{h}", [NC, 128, 128], bf16)
                 for h in range(2)] for g in range(B)]
    a2a_dram = [[nc.dram_tensor(f"ctxa2a{g}h{h}", [NC, 128, 128], bf16)
                 for h in range(2)] for g in range(B)]

    with TileContext(nc) as tc:
        with (
            tc.tile_pool(name="const", bufs=1) as cpool,
            tc.tile_pool(name="big", bufs=1) as bigpool,
            tc.tile_pool(name="xstream", bufs=2) as xpool,
            tc.tile_pool(name="vt", bufs=2) as vtpool,
            tc.tile_pool(name="exp", bufs=1) as epool,
            tc.tile_pool(name="small", bufs=2) as spool,
            tc.tile_pool(name="ag", bufs=3) as agpool,
            tc.tile_pool(name="outp", bufs=2) as opool,
            tc.tile_pool(name="psA", bufs=2, space="PSUM") as psA,   # 2x [128,1024]
            tc.tile_pool(name="psB", bufs=4, space="PSUM") as psB,   # 4x [128,512]
        ):
            # ---- constants / weights to SBUF ----
            # wqk + round-0 x gate the whole kernel: queue them first, in
            # arrival-need order, all per-partition-contiguous in DRAM.
            wqk_sb = cpool.tile([128, KC, 256], bf16, tag="wqk")
            x0 = xpool.tile([128, KC, 1024], bf16, tag="xt", name="x0")
            x1 = xpool.tile([128, KC, 1024], bf16, tag="xt", name="x1")
            # per-queue DMA bandwidth saturates well below HBM rate, so the
            # x stream is split across the sync and scalar queues
            # fine-grained startup chunks round-robined over the three
            # DMA-capable queues: the first matmul needs only wqk kk0-1 +
            # x0 kk0, and the aggregate stream keeps round 0/1 fed densely
            qs = [nc.sync, nc.scalar, nc.gpsimd]
            nc.sync.dma_start(wqk_sb[:, 0:2, :], wqk[:, 0:2, :])
            nc.scalar.dma_start(wqk_sb[:, 2:4, :], wqk[:, 2:4, :])
            nc.gpsimd.dma_start(wqk_sb[:, 4:6, :], wqk[:, 4:6, :])
            nc.scalar.dma_start(wqk_sb[:, 6:8, :], wqk[:, 6:8, :])
            for kk in range(KC):
                qs[kk % 3].dma_start(x0[:, kk:kk + 1, :],
                                     xr_d[0][:, kk:kk + 1, :])
            for c in range(4):
                qs[c % 3].dma_start(x1[:, 2 * c:2 * c + 2, :],
                                    xr_d[1][:, 2 * c:2 * c + 2, :])
            wv_sb = cpool.tile([128, KC, 128], bf16, tag="wv")
            nc.scalar.dma_start(wv_sb[:], wv[:])
            bqk_sb = cpool.tile([128, 2], f32, tag="bqk")
            nc.scalar.dma_start(bqk_sb[:], bqk[:])
            bv_sb = cpool.tile([128, 1], f32, tag="bv")
            nc.scalar.dma_start(bv_sb[:], bv[:])
            mask_sb = cpool.tile([128, 128], bf16, tag="mask")
            nc.scalar.dma_start(mask_sb[:], mask[:])
            ident_sb = cpool.tile([128, 128], bf16, tag="ident")
            nc.scalar.dma_start(ident_sb[:], ident[:])
            wo_sb = cpool.tile([128, KC, D], bf16, tag="wo")
            bo_sb = cpool.tile([128, D], bf16, tag="bo")

            # ---- persistent activations ----
            # q stored twice, one copy per head with the other head's rows
            # zeroed: scores matmuls contract K=128 so the PE stays in
            # 128-row tile mode (64-row mode entries drain the array)
            qTp = [bigpool.tile([128, BS], bf16, tag=f"qT{hl}",
                                name=f"qT{hl}")
                   for hl in range(HPC)]
            kT_sb = bigpool.tile([128, BS], bf16, tag="kT")
            # v natural layout: [kpos, tile, head, 65] with ones at col 64
            v_sb = bigpool.tile([128, BS // 128, HPC, 65], bf16, tag="v")
            ctxT_sb = bigpool.tile([128, BS], bf16, tag="ctxT")

            nc.vector.memset(v_sb[:, :, :, 64:65], 1.0)
            nc.vector.memset(qTp[0][64:128, :], 0.0)
            nc.vector.memset(qTp[1][0:64, :], 0.0)

            vt_tiles = {}
            exp_tiles = {}

            # ---- qkv projection: rounds of 1024 tokens ----
            x_tiles = {0: x0, 1: x1}

            def emit_x_load(r):
                xt = xpool.tile([128, KC, 1024], bf16, tag="xt", name="xt")
                x_tiles[r] = xt
                eng2 = nc.scalar if r == 2 else nc.gpsimd
                for c in range(4):
                    eng = nc.sync if c % 2 == 0 else eng2
                    eng.dma_start(xt[:, 2 * c:2 * c + 2, :],
                                  xr_d[r][:, 2 * c:2 * c + 2, :])

            def emit_qkv_round(r):
                lo = r * 1024
                xt = x_tiles[r]

                ps_q = psA.tile([128, 1024], f32, tag="psA", name="ps_q")
                ps_k = psA.tile([128, 1024], f32, tag="psA", name="ps_k")
                ps_v0 = psB.tile([128, 512], f32, tag="psB", name="ps_v0")
                ps_v1 = psB.tile([128, 512], f32, tag="psB", name="ps_v1")
                for kk in range(KC):
                    nc.tensor.matmul(ps_q[:, 0:512], lhsT=wqk_sb[:, kk, 0:128],
                                     rhs=xt[:, kk, 0:512],
                                     start=(kk == 0), stop=(kk == KC - 1))
                    nc.tensor.matmul(ps_q[:, 512:1024], lhsT=wqk_sb[:, kk, 0:128],
                                     rhs=xt[:, kk, 512:1024],
                                     start=(kk == 0), stop=(kk == KC - 1))
                for kk in range(KC):
                    nc.tensor.matmul(ps_k[:, 0:512], lhsT=wqk_sb[:, kk, 128:256],
                                     rhs=xt[:, kk, 0:512],
                                     start=(kk == 0), stop=(kk == KC - 1))
                    nc.tensor.matmul(ps_k[:, 512:1024], lhsT=wqk_sb[:, kk, 128:256],
                                     rhs=xt[:, kk, 512:1024],
                                     start=(kk == 0), stop=(kk == KC - 1))
                for kk in range(KC):
                    nc.tensor.matmul(ps_v0, lhsT=wv_sb[:, kk, :],
                                     rhs=xt[:, kk, 0:512],
                                     start=(kk == 0), stop=(kk == KC - 1))
                    nc.tensor.matmul(ps_v1, lhsT=wv_sb[:, kk, :],
                                     rhs=xt[:, kk, 512:1024],
                                     start=(kk == 0), stop=(kk == KC - 1))
                vt = vtpool.tile([128, 1024], bf16, tag="vT", name="vt")
                vt_tiles[r] = vt
                nc.vector.tensor_scalar_add(qTp[0][0:64, lo:lo + 1024],
                                            ps_q[0:64, :], bqk_sb[0:64, 0:1])
                nc.vector.tensor_scalar_add(qTp[1][64:128, lo:lo + 1024],
                                            ps_q[64:128, :],
                                            bqk_sb[64:128, 0:1])
                nc.vector.tensor_scalar_add(kT_sb[:, lo:lo + 1024], ps_k,
                                            bqk_sb[:, 1:2])
                nc.vector.tensor_scalar_add(vt[:, 0:512], ps_v0,
                                            bv_sb[:, 0:1])
                nc.vector.tensor_scalar_add(vt[:, 512:1024], ps_v1,
                                            bv_sb[:, 0:1])

            # v^T [feat, tok] -> natural [tok, feat] via PE transposes,
            # 8 tiles packed per PSUM tile, drained by one strided DVE copy.
            def emit_v_transposes(r):
                vt = vt_tiles.pop(r)
                pack = psA.tile([128, 8, HPC, 64], bf16, tag="psA", name="tpack")
                for t8 in range(8):
                    c0 = t8 * 128
                    nc.tensor.transpose(pack[:, t8], vt[:, c0:c0 + 128],
                                        ident_sb[:])
                nc.vector.tensor_copy(v_sb[:, r * 8:(r + 1) * 8, :, 0:64],
                                      pack[:])

            # ---- collectives + output projection ----
            def emit_a2a(b, h):
                cols = slice(b * S + h * 1024, b * S + (h + 1) * 1024)
                nc.gpsimd.dma_start(
                    ctx_dram[b][h].rearrange("j p s -> p j s"),
                    ctxT_sb[:, cols])
                nc.gpsimd.collective_compute(
                    "AllToAll",
                    mybir.AluOpType.bypass,
                    replica_groups=[list(range(NC))],
                    ins=[ctx_dram[b][h][:]],
                    outs=[a2a_dram[b][h][:]],
                )

            def load_ctxag(g):
                b, half = g // 2, g % 2
                ctxag_sb = agpool.tile([128, NC, 128], bf16, tag="ctxag",
                                       name="ctxag_sb")
                src_v = a2a_dram[b][half].rearrange("j p s -> p j s")
                nc.sync.dma_start(ctxag_sb[:, 0:4, :], src_v[:, 0:4, :])
                nc.sync.dma_start(ctxag_sb[:, 4:8, :], src_v[:, 4:8, :])
                return ctxag_sb

            def emit_proj(g, ctxag_sb=None):
                if ctxag_sb is None:
                    ctxag_sb = load_ctxag(g)
                ps_o = psA.tile([128, 1024], f32, tag="psA", name="ps_o")
                # sequential halves: half-0's bias-add + store overlap
                # half-1's matmuls, shortening the tail after the last a2a
                for half2 in range(2):
                    cs = slice(512 * half2, 512 * half2 + 512)
                    for k in range(NC):
                        nc.tensor.matmul(ps_o[:, cs],
                                         lhsT=ctxag_sb[:, k, :],
                                         rhs=wo_sb[:, k, cs],
                                         start=(k == 0), stop=(k == NC - 1))
                    ot = opool.tile([128, 512], f32, tag="ot")
                    nc.vector.tensor_tensor(ot[:], ps_o[:, cs],
                                            bo_sb[:, cs], ALU.add)
                    nc.sync.dma_start(out[g * 128:(g + 1) * 128, cs], ot[:])

            # ---- AV + normalize, emitted as fine-grained filler ----
            # Returns a list of closures: PE matmul chunks (interleaved
            # between the next window's scores pieces to keep the in-order
            # PE busy) followed by one normalize closure per head (vector/
            # gpsimd work, order-free).
            def build_av_steps(b, j):
                steps = []
                nkt = 4 * (j + 1)
                for hl in range(HPC):
                    exp_j = exp_tiles[(j, hl)]
                    ps_c = psB.tile([128, 512], f32, tag="psB", name="ps_c")
                    # full k-tiles at N=512, then ragged diagonal tiles at
                    # N=512-128*o (the causally-invalid prefix is skipped)
                    mms = [(tt, 0) for tt in range(4 * j)]
                    mms += [(4 * j + o, 128 * o) for o in range(4)]

                    def mk_mm(pair, first, last, hl=hl, exp_j=exp_j,
                              ps_c=ps_c):
                        def go():
                            for i, (tt, q0) in enumerate(pair):
                                nc.tensor.matmul(
                                    ps_c[:65, q0:512],
                                    lhsT=v_sb[:, b * NKT + tt, hl, :],
                                    rhs=exp_j[:, tt * 512 + q0:
                                              (tt + 1) * 512],
                                    start=(first and i == 0),
                                    stop=(last and i == len(pair) - 1))
                        return go

                    for ci in range(0, len(mms), 2):
                        pair = mms[ci:ci + 2]
                        steps.append(mk_mm(pair, ci == 0,
                                           ci + 2 >= len(mms)))

                    def mk_norm(hl=hl, ps_c=ps_c):
                        def go():
                            hp = slice(64 * hl, 64 * hl + 64)
                            # den/recip read PSUM directly, in parallel with
                            # the stage copy — shortens the serial chain to
                            # the a2a trigger by ~3us
                            den = spool.tile([1, 512], f32, tag="den")
                            nc.vector.tensor_copy(den[:], ps_c[64:65, :])
                            recip = spool.tile([1, 512], f32, tag="recip")
                            nc.vector.reciprocal_approx_fast(out=recip[:],
                                                             in_=den[:])
                            stage = spool.tile([64, 512], f32, tag="stage",
                                               bufs=2)
                            nc.vector.tensor_copy(stage[:], ps_c[0:64, :])
                            bcast = spool.tile([64, 512], f32, tag="bcast",
                                               bufs=2)
                            nc.gpsimd.partition_broadcast(bcast[:], recip[:])
                            cs = slice(b * S + j * 512,
                                       b * S + (j + 1) * 512)
                            nc.vector.tensor_tensor(ctxT_sb[hp, cs],
                                                    stage[:], bcast[:],
                                                    ALU.mult)
                        return go

                    steps.append(mk_norm())
                return steps

            def emit_pe_warm(n):
                # dummy matmuls that keep the PE busy (and at max pstate)
                # while the final collective's peer-wait + transfer drain;
                # results are never read
                for i in range(n):
                    ps_w = psA.tile([128, 1024], f32, tag="psA",
                                    name="ps_warm")
                    nc.tensor.matmul(ps_w[:, 0:512],
                                     lhsT=wqk_sb[:, 0, 0:128],
                                     rhs=qT_sb[:, 0:512],
                                     start=True, stop=True)

            filler = []

            def run_filler(n):
                for _ in range(min(n, len(filler))):
                    filler.pop(0)()

            def drain_filler(before_last=None):
                while filler:
                    if before_last is not None and len(filler) == 1:
                        before_last()
                        before_last = None
                    filler.pop(0)()

            def emit_window(b, j):
                nkt = 4 * (j + 1)
                exp_js = []
                for hl in range(HPC):
                    t = epool.tile([128, nkt * 512], bf16,
                                   tag=f"expj{j}h{hl}", name="exp_j")
                    exp_tiles[(j, hl)] = t
                    exp_js.append(t)
                # Scores pieces: off-diagonal k-tiles (full 512-q) in pairs,
                # then the 4 diagonal tiles (ragged: tile 4j+o covers the
                # last 512-128*o q columns) packed into two PSUM tiles.
                win = b * S + j * 512
                pieces = []
                tt = 0
                while tt < 4 * j:
                    npc = min(2, 4 * j - tt)
                    pieces.append([(tt + i, 0) for i in range(npc)])
                    tt += npc
                pieces.append([(4 * j, 0), (4 * j + 1, 128)])
                pieces.append([(4 * j + 2, 256), (4 * j + 3, 384)])
                n_pieces = len(pieces)
                for pi, piece in enumerate(pieces):
                    ps_h = [psA.tile([128, 1024], f32, tag="psA",
                                     name="ps_sc")
                            for _ in range(HPC)]
                    # pack spans so no matmul output crosses a 512-col
                    # (2KB) PSUM bank boundary
                    col = 0
                    spans = []
                    for (tile_idx, qoff) in piece:
                        w = 512 - qoff
                        if col // 512 != (col + w - 1) // 512:
                            col = ((col + 511) // 512) * 512
                        spans.append((tile_idx, qoff, col, w))
                        col += w
                    for (tile_idx, qoff, c0, w) in spans:
                        kt = b * S + tile_idx * 128
                        for hl in range(HPC):
                            nc.tensor.matmul(
                                ps_h[hl][:, c0:c0 + w],
                                lhsT=kT_sb[:, kt:kt + 128],
                                rhs=qTp[hl][:, win + qoff:win + 512],
                                start=True, stop=True)
                    # exp: one ACT op per head per src/dst-contiguous run
                    for hl in range(HPC):
                        run = []
                        for (tile_idx, qoff, c0, w) in spans:
                            dst = tile_idx * 512 + qoff
                            if run and run[-1][1] + run[-1][2] == dst \
                                    and run[-1][0] + run[-1][2] == c0:
                                run[-1] = (run[-1][0], run[-1][1],
                                           run[-1][2] + w)
                            else:
                                run.append((c0, dst, w))
                        for (c0, dst, w) in run:
                            nc.scalar.activation(
                                exp_js[hl][:, dst:dst + w],
                                ps_h[hl][:, c0:c0 + w], ACTF.Exp)
                    # interleave AV filler of the previous window so the
                    # in-order PE has work while Scalar drains exp. Coarse
                    # grain (every 2nd piece): scores run in 64-row PE tile
                    # mode, AV in 128-row mode, and each mode switch drains
                    # the PE array
                    if filler and (pi % 2 == 1 or pi == n_pieces - 1):
                        rem = (n_pieces - pi + 1) // 2
                        per = -(-len(filler) // max(rem, 1))
                        run_filler(per)
                # triangular causal mask on each diagonal tile's first
                # 128 valid columns
                for hl in range(HPC):
                    for o in range(4):
                        lo = (4 * j + o) * 512 + 128 * o
                        nc.vector.tensor_tensor(exp_js[hl][:, lo:lo + 128],
                                                exp_js[hl][:, lo:lo + 128],
                                                mask_sb[:],
                                                ALU.mult)
                drain_filler()
                filler.extend(build_av_steps(b, j))

            scope1 = nc.named_scope("qkv"); scope1.__enter__()
            emit_qkv_round(0)
            emit_qkv_round(1)
            emit_v_transposes(0)
            emit_v_transposes(1)
            scope1.__exit__(None, None, None)

            scope2 = nc.named_scope("attn"); scope2.__enter__()
            # batch-0 windows run consecutively so the first two collectives
            # fire as early as possible — the CC queue (4 meshes x ~20-30us,
            # strictly serial) is the tail's critical path. qkv rounds 2/3
            # slot in after batch 0; their x chunks are pre-issued here.
            emit_window(0, 0)
            emit_x_load(2)
            nc.scalar.dma_start(wo_sb[:], wo[:])
            nc.scalar.dma_start(bo_sb[:], bo[:])
            emit_window(0, 1)
            emit_x_load(3)
            emit_window(0, 2)
            emit_a2a(0, 0)             # data (norms of 0,0/0,1) ready
            emit_window(0, 3)
            drain_filler()             # AV(0,3) + norms
            emit_a2a(0, 1)
            emit_qkv_round(2)
            emit_v_transposes(2)
            emit_qkv_round(3)
            emit_v_transposes(3)
            emit_window(1, 0)
            emit_window(1, 1)
            ag0 = load_ctxag(0)
            emit_window(1, 2)
            emit_a2a(1, 0)
            ag1 = load_ctxag(1)
            emit_window(1, 3)
            drain_filler()             # AV(1,3) + norms
            ag2 = load_ctxag(2)
            emit_proj(0, ag0)
            emit_proj(1, ag1)
            emit_proj(2, ag2)
            emit_a2a(1, 1)
            ag3 = load_ctxag(3)
            emit_proj(3, ag3)
            scope2.__exit__(None, None, None)

    nc.compile()
    return nc


def _prep_inputs(x, Wqkv, bqkv, Wo, bo):
    x = np.asarray(x, dtype=np.float32)
    Wqkv = np.asarray(Wqkv, dtype=np.float32)
    bqkv = np.asarray(bqkv, dtype=np.float32)
    Wo = np.asarray(Wo, dtype=np.float32)
    bo = np.asarray(bo, dtype=np.float32)

    xT = x.reshape(BS, D).T.astype(BF16)               # [D, BS]
    # per-round per-partition-contiguous tiles: xr[r][p, ko, t] =
    # xT[ko*128+p, r*1024+t]
    xr = np.ascontiguousarray(
        xT.reshape(KC, 128, NR, 1024).transpose(2, 1, 0, 3))
    wo_t = np.ascontiguousarray(
        Wo.astype(BF16).reshape(KC, 128, D).transpose(1, 0, 2))
    bo_t = np.tile(bo.astype(BF16), (128, 1))

    kp = np.arange(128)[:, None]
    u = np.arange(128)[None, :]
    mask = (u >= kp).astype(BF16)
    ident = np.eye(128, dtype=BF16)

    scale = np.float32(1.0 / np.sqrt(HD))

    # Wqkv columns per head h: q = 192h..+64, k = +64, v = +128
    W3 = Wqkv.reshape(D, H, 3, HD)
    b3 = bqkv.reshape(H, 3, HD)

    in_maps = []
    for c in range(NC):
        hs = [HPC * c + i for i in range(HPC)]
        wq = np.concatenate([W3[:, h, 0, :] for h in hs], axis=1) * scale
        wk = np.concatenate([W3[:, h, 1, :] for h in hs], axis=1)
        wv_ = np.concatenate([W3[:, h, 2, :] for h in hs], axis=1)
        bq = np.concatenate([b3[h, 0, :] for h in hs]) * scale
        bk = np.concatenate([b3[h, 1, :] for h in hs])
        bv_ = np.concatenate([b3[h, 2, :] for h in hs])
        wqk_c = np.concatenate([wq, wk], axis=1).astype(BF16)   # [D, 256]
        wqk_t = np.ascontiguousarray(
            wqk_c.reshape(KC, 128, 256).transpose(1, 0, 2))
        wv_t = np.ascontiguousarray(
            wv_.astype(BF16).reshape(KC, 128, 128).transpose(1, 0, 2))
        m = {
            "wqk": wqk_t,
            "wv": wv_t,
            "wo": wo_t,
            "bqk": np.ascontiguousarray(
                np.stack([bq, bk], axis=1)).astype(np.float32),
            "bv": bv_.astype(np.float32).reshape(128, 1),
            "bo": bo_t,
            "mask": mask,
            "ident": ident,
        }
        for r in range(NR):
            m[f"xr{r}"] = xr[r]
        in_maps.append(m)
    return in_maps


def run(x, Wqkv, bqkv, Wo, bo, trace=False):
    from concourse.bass_utils import run_bass_kernel_spmd

    if "nc" not in _CACHE:
        _CACHE["nc"] = _build_program()
    nc = _CACHE["nc"]
    in_maps = _prep_inputs(x, Wqkv, bqkv, Wo, bo)
    res = run_bass_kernel_spmd(nc, in_maps, list(range(NC)), trace=trace)
    # core c returns [512, D]: 4 chunks of 128 rows: (b0 rows 128c..),
    # (b0 rows 1024+128c..), (b1 rows 128c..), (b1 rows 1024+128c..)
    full = np.empty((B, S, D), dtype=np.float32)
    for c in range(NC):
        r = res.results[c]["out"]
        for g in range(4):
            b, half = g // 2, g % 2
            lo = half * 1024 + 128 * c
            full[b, lo:lo + 128, :] = r[g * 128:(g + 1) * 128, :]
    return full, res


def kernel(x, Wqkv, bqkv, Wo, bo):
    out, _ = run(x, Wqkv, bqkv, Wo, bo)
    return out


# revision 19
# speedup vs baseline: 1.2140x; 1.0267x over previous
"""Trainium2 Bass kernel for causal multi-head attention.

Problem: B=2, S=2048, D=1024, H=16 heads (hd=64), fp32 in/out.
  qkv = x @ Wqkv + bqkv ; per-head causal softmax attention ; out = ctx @ Wo + bo

Sharding (8 NeuronCores): tensor-parallel over heads — 2 heads per core.
Each core computes q/k/v projections for its 2 heads (both batches), causal
attention, and its ctx^T slice [128 feat, B*S]. Four AllToAll collectives
(one per (batch, half), 256KB each) redistribute ctx^T from head-sharded to
row-sharded; each core then projects 128 rows per chunk with the full Wo.
Host reassembles the row slices.

v4 vs v2:
- All big inputs (x rounds, wqk, wv, wo) pre-tiled host-side into
  per-partition-contiguous DRAM layouts: DMA issue drops from ~1-8us to
  ~0.2us each and transfers run at full HBM rate; first matmul starts ~6us
  earlier.
- Ragged AV: the attn@v accumulation skips the causally-invalid q-ranges of
  the diagonal k-tiles (N=512-128*o) instead of multiplying zeroed exp.
  Saves ~25% of AV matmul columns and drops the exp-tile zero memsets.
- Fine-grained PE interleave: the AV matmuls of window j-1 are emitted
  between the scores pieces of window j (the PE is in-order, so filler must
  be interleaved in emission order). The PE no longer stalls while the
  Scalar engine drains exp.
- The per-batch AllToAll is split into four per-(batch, half) collectives,
  each fired as soon as its two windows' ctx is normalized; projections are
  spread between late windows so only a 256KB collective + one projection
  remain in the tail.

Numerics: bf16 matmul operands, fp32 PSUM accumulation. Softmax uses
exp without max-subtraction (scores are ~N(0,1) after the folded 1/sqrt(hd)
scale; |s| < ~8 so fp32 exp/sums are safe). The softmax denominator comes
for free as a ones-column appended to v in the attn@v matmul.
"""

import numpy as np
import ml_dtypes

B, S, D, H, NC = 2, 2048, 1024, 16, 8
HD = D // H            # 64
HPC = H // NC          # 2 heads per core
BS = B * S             # 4096
RPB = S // NC          # 256 output rows per core per batch
KC = D // 128          # 8 contraction chunks
NR = 4                 # qkv rounds of 1024 tokens
NKT = S // 128         # 16 k-tiles (128) per batch

BF16 = ml_dtypes.bfloat16

_CACHE = {}


def _build_program():
    import concourse.bass as bass
    import concourse.mybir as mybir
    from concourse import bacc
    from concourse.tile import TileContext

    dt = mybir.dt
    f32, bf16 = dt.float32, dt.bfloat16
    ALU = mybir.AluOpType
    ACTF = mybir.ActivationFunctionType

    nc = bacc.Bacc("TRN2", target_bir_lowering=False, debug=False, num_devices=NC)

    xr_d = [nc.dram_tensor(f"xr{r}", [128, KC, 1024], bf16,
                           kind="ExternalInput") for r in range(NR)]
    wqk = nc.dram_tensor("wqk", [128, KC, 256], bf16, kind="ExternalInput")
    wv = nc.dram_tensor("wv", [128, KC, 128], bf16, kind="ExternalInput")
    wo = nc.dram_tensor("wo", [128, KC, D], bf16, kind="ExternalInput")
    bqk = nc.dram_tensor("bqk", [128, 2], f32, kind="ExternalInput")
    bv = nc.dram_tensor("bv", [128, 1], f32, kind="ExternalInput")
    bo = nc.dram_tensor("bo", [128, D], bf16, kind="ExternalInput")
    mask = nc.dram_tensor("mask", [128, 128], bf16, kind="ExternalInput")
    ident = nc.dram_tensor("ident", [128, 128], bf16, kind="ExternalInput")
    out = nc.dram_tensor("out", [2 * RPB, D], f32, kind="ExternalOutput")

    # collective buffers: one AllToAll per (batch, half). Shard j of the
    # send buffer = [our 128 feats, core j's 128 q rows of this half];
    # the received shard j = [core j's 128 feats, our 128 rows].
    ctx_dram = [[nc.dram_tensor(f"ctxb{g}h{h}", [NC, 128, 128], bf16)
                 for h in range(2)] for g in range(B)]
    a2a_dram = [[nc.dram_tensor(f"ctxa2a{g}h{h}", [NC, 128, 128], bf16)
                 for h in range(2)] for g in range(B)]

    with TileContext(nc) as tc:
        with (
            tc.tile_pool(name="const", bufs=1) as cpool,
            tc.tile_pool(name="big", bufs=1) as bigpool,
            tc.tile_pool(name="xstream", bufs=2) as xpool,
            tc.tile_pool(name="vt", bufs=2) as vtpool,
            tc.tile_pool(name="exp", bufs=1) as epool,
            tc.tile_pool(name="small", bufs=2) as spool,
            tc.tile_pool(name="ag", bufs=3) as agpool,
            tc.tile_pool(name="outp", bufs=2) as opool,
            tc.tile_pool(name="psA", bufs=2, space="PSUM") as psA,   # 2x [128,1024]
            tc.tile_pool(name="psB", bufs=4, space="PSUM") as psB,   # 4x [128,512]
        ):
            # ---- constants / weights to SBUF ----
            # wqk + round-0 x gate the whole kernel: queue them first, in
            # arrival-need order, all per-partition-contiguous in DRAM.
            wqk_sb = cpool.tile([128, KC, 256], bf16, tag="wqk")
            x0 = xpool.tile([128, KC, 1024], bf16, tag="xt", name="x0")
            x1 = xpool.tile([128, KC, 1024], bf16, tag="xt", name="x1")
            # per-queue DMA bandwidth saturates well below HBM rate, so the
            # x stream is split across the sync and scalar queues
            # fine-grained startup chunks round-robined over the three
            # DMA-capable queues: the first matmul needs only wqk kk0-1 +
            # x0 kk0, and the aggregate stream keeps round 0/1 fed densely
            qs = [nc.sync, nc.scalar, nc.gpsimd]
            nc.sync.dma_start(wqk_sb[:, 0:2, :], wqk[:, 0:2, :])
            nc.scalar.dma_start(wqk_sb[:, 2:4, :], wqk[:, 2:4, :])
            nc.gpsimd.dma_start(wqk_sb[:, 4:6, :], wqk[:, 4:6, :])
            nc.scalar.dma_start(wqk_sb[:, 6:8, :], wqk[:, 6:8, :])
            for kk in range(KC):
                qs[kk % 3].dma_start(x0[:, kk:kk + 1, :],
                                     xr_d[0][:, kk:kk + 1, :])
            for c in range(4):
                qs[c % 3].dma_start(x1[:, 2 * c:2 * c + 2, :],
                                    xr_d[1][:, 2 * c:2 * c + 2, :])
            wv_sb = cpool.tile([128, KC, 128], bf16, tag="wv")
            nc.scalar.dma_start(wv_sb[:], wv[:])
            bqk_sb = cpool.tile([128, 2], f32, tag="bqk")
            nc.scalar.dma_start(bqk_sb[:], bqk[:])
            bv_sb = cpool.tile([128, 1], f32, tag="bv")
            nc.scalar.dma_start(bv_sb[:], bv[:])
            mask_sb = cpool.tile([128, 128], bf16, tag="mask")
            nc.scalar.dma_start(mask_sb[:], mask[:])
            ident_sb = cpool.tile([128, 128], bf16, tag="ident")
            nc.scalar.dma_start(ident_sb[:], ident[:])
            wo_sb = cpool.tile([128, KC, D], bf16, tag="wo")
            bo_sb = cpool.tile([128, D], bf16, tag="bo")

            # ---- persistent activations ----
            # q stored twice, one copy per head with the other head's rows
            # zeroed: scores matmuls contract K=128 so the PE stays in
            # 128-row tile mode (64-row mode entries drain the array)
            qTp = [bigpool.tile([128, BS], bf16, tag=f"qT{hl}",
                                name=f"qT{hl}")
                   for hl in range(HPC)]
            kT_sb = bigpool.tile([128, BS], bf16, tag="kT")
            # v natural layout: [kpos, tile, head, 65] with ones at col 64
            v_sb = bigpool.tile([128, BS // 128, HPC, 65], bf16, tag="v")
            ctxT_sb = bigpool.tile([128, BS], bf16, tag="ctxT")

            nc.vector.memset(v_sb[:, :, :, 64:65], 1.0)
            nc.vector.memset(qTp[0][64:128, :], 0.0)
            nc.vector.memset(qTp[1][0:64, :], 0.0)

            vt_tiles = {}
            exp_tiles = {}

            # ---- qkv projection: rounds of 1024 tokens ----
            x_tiles = {0: x0, 1: x1}

            def emit_x_load(r):
                xt = xpool.tile([128, KC, 1024], bf16, tag="xt", name="xt")
                x_tiles[r] = xt
                eng2 = nc.scalar if r == 2 else nc.gpsimd
                for c in range(4):
                    eng = nc.sync if c % 2 == 0 else eng2
                    eng.dma_start(xt[:, 2 * c:2 * c + 2, :],
                                  xr_d[r][:, 2 * c:2 * c + 2, :])

            def emit_qkv_round(r):
                lo = r * 1024
                xt = x_tiles[r]

                ps_q = psA.tile([128, 1024], f32, tag="psA", name="ps_q")
                ps_k = psA.tile([128, 1024], f32, tag="psA", name="ps_k")
                ps_v0 = psB.tile([128, 512], f32, tag="psB", name="ps_v0")
                ps_v1 = psB.tile([128, 512], f32, tag="psB", name="ps_v1")
                for kk in range(KC):
                    nc.tensor.matmul(ps_q[:, 0:512], lhsT=wqk_sb[:, kk, 0:128],
                                     rhs=xt[:, kk, 0:512],
                                     start=(kk == 0), stop=(kk == KC - 1))
                    nc.tensor.matmul(ps_q[:, 512:1024], lhsT=wqk_sb[:, kk, 0:128],
                                     rhs=xt[:, kk, 512:1024],
                                     start=(kk == 0), stop=(kk == KC - 1))
                for kk in range(KC):
                    nc.tensor.matmul(ps_k[:, 0:512], lhsT=wqk_sb[:, kk, 128:256],
                                     rhs=xt[:, kk, 0:512],
                                     start=(kk == 0), stop=(kk == KC - 1))
                    nc.tensor.matmul(ps_k[:, 512:1024], lhsT=wqk_sb[:, kk, 128:256],
                                     rhs=xt[:, kk, 512:1024],
                                     start=(kk == 0), stop=(kk == KC - 1))
                for kk in range(KC):
                    nc.tensor.matmul(ps_v0, lhsT=wv_sb[:, kk, :],
                                     rhs=xt[:, kk, 0:512],
                                     start=(kk == 0), stop=(kk == KC - 1))
                    nc.tensor.matmul(ps_v1, lhsT=wv_sb[:, kk, :],
                                     rhs=xt[:, kk, 512:1024],
                                     start=(kk == 0), stop=(kk == KC - 1))
                vt = vtpool.tile([128, 1024], bf16, tag="vT", name="vt")
                vt_tiles[r] = vt
                nc.vector.tensor_scalar_add(qTp[0][0:64, lo:lo + 1024],
                                            ps_q[0:64, :], bqk_sb[0:64, 0:1])
                nc.vector.tensor_scalar_add(qTp[1][64:128, lo:lo + 1024],
                                            ps_q[64:128, :],
                                            bqk_sb[64:128, 0:1])
                nc.vector.tensor_scalar_add(kT_sb[:, lo:lo + 1024], ps_k,
                                            bqk_sb[:, 1:2])
                nc.vector.tensor_scalar_add(vt[:, 0:512], ps_v0,
                                            bv_sb[:, 0:1])
                nc.vector.tensor_scalar_add(vt[:, 512:1024], ps_v1,
                                            bv_sb[:, 0:1])

            # v^T [feat, tok] -> natural [tok, feat] via PE transposes,
            # 8 tiles packed per PSUM tile, drained by one strided DVE copy.
            def emit_v_transposes(r):
                vt = vt_tiles.pop(r)
                pack = psA.tile([128, 8, HPC, 64], bf16, tag="psA", name="tpack")
                for t8 in range(8):
                    c0 = t8 * 128
                    nc.tensor.transpose(pack[:, t8], vt[:, c0:c0 + 128],
                                        ident_sb[:])
                nc.vector.tensor_copy(v_sb[:, r * 8:(r + 1) * 8, :, 0:64],
                                      pack[:])

            # ---- collectives + output projection ----
            def emit_a2a(b, h):
                # store split per source window: the first window's half was
                # normalized a window earlier and transfers immediately,
                # leaving only 128KB on the trigger's critical path
                dst = ctx_dram[b][h].rearrange("j p s -> p j s")
                c0 = b * S + h * 1024
                nc.gpsimd.dma_start(dst[:, 0:4, :], ctxT_sb[:, c0:c0 + 512])
                nc.gpsimd.dma_start(dst[:, 4:8, :],
                                    ctxT_sb[:, c0 + 512:c0 + 1024])
                nc.gpsimd.collective_compute(
                    "AllToAll",
                    mybir.AluOpType.bypass,
                    replica_groups=[list(range(NC))],
                    ins=[ctx_dram[b][h][:]],
                    outs=[a2a_dram[b][h][:]],
                )

            def load_ctxag(g):
                b, half = g // 2, g % 2
                ctxag_sb = agpool.tile([128, NC, 128], bf16, tag="ctxag",
                                       name="ctxag_sb")
                src_v = a2a_dram[b][half].rearrange("j p s -> p j s")
                nc.sync.dma_start(ctxag_sb[:, 0:4, :], src_v[:, 0:4, :])
                nc.sync.dma_start(ctxag_sb[:, 4:8, :], src_v[:, 4:8, :])
                return ctxag_sb

            def emit_proj(g, ctxag_sb=None):
                if ctxag_sb is None:
                    ctxag_sb = load_ctxag(g)
                ps_o = psA.tile([128, 1024], f32, tag="psA", name="ps_o")
                # sequential halves: half-0's bias-add + store overlap
                # half-1's matmuls, shortening the tail after the last a2a
                for half2 in range(2):
                    cs = slice(512 * half2, 512 * half2 + 512)
                    for k in range(NC):
                        nc.tensor.matmul(ps_o[:, cs],
                                         lhsT=ctxag_sb[:, k, :],
                                         rhs=wo_sb[:, k, cs],
                                         start=(k == 0), stop=(k == NC - 1))
                    ot = opool.tile([128, 512], f32, tag="ot")
                    nc.vector.tensor_tensor(ot[:], ps_o[:, cs],
                                            bo_sb[:, cs], ALU.add)
                    nc.sync.dma_start(out[g * 128:(g + 1) * 128, cs], ot[:])

            # ---- AV + normalize, emitted as fine-grained filler ----
            # Returns a list of closures: PE matmul chunks (interleaved
            # between the next window's scores pieces to keep the in-order
            # PE busy) followed by one normalize closure per head (vector/
            # gpsimd work, order-free).
            def build_av_steps(b, j):
                steps = []
                nkt = 4 * (j + 1)
                for hl in range(HPC):
                    exp_j = exp_tiles[(j, hl)]
                    ps_c = psB.tile([128, 512], f32, tag="psB", name="ps_c")
                    # full k-tiles at N=512, then ragged diagonal tiles at
                    # N=512-128*o (the causally-invalid prefix is skipped)
                    mms = [(tt, 0) for tt in range(4 * j)]
                    mms += [(4 * j + o, 128 * o) for o in range(4)]

                    def mk_mm(pair, first, last, hl=hl, exp_j=exp_j,
                              ps_c=ps_c):
                        def go():
                            for i, (tt, q0) in enumerate(pair):
                                nc.tensor.matmul(
                                    ps_c[:65, q0:512],
                                    lhsT=v_sb[:, b * NKT + tt, hl, :],
                                    rhs=exp_j[:, tt * 512 + q0:
                                              (tt + 1) * 512],
                                    start=(first and i == 0),
                                    stop=(last and i == len(pair) - 1))
                        return go

                    for ci in range(0, len(mms), 2):
                        pair = mms[ci:ci + 2]
                        steps.append(mk_mm(pair, ci == 0,
                                           ci + 2 >= len(mms)))

                    def mk_norm(hl=hl, ps_c=ps_c):
                        def go():
                            hp = slice(64 * hl, 64 * hl + 64)
                            # den/recip read PSUM directly, in parallel with
                            # the stage copy — shortens the serial chain to
                            # the a2a trigger by ~3us
                            den = spool.tile([1, 512], f32, tag="den")
                            nc.vector.tensor_copy(den[:], ps_c[64:65, :])
                            recip = spool.tile([1, 512], f32, tag="recip")
                            nc.vector.reciprocal_approx_fast(out=recip[:],
                                                             in_=den[:])
                            stage = spool.tile([64, 512], f32, tag="stage",
                                               bufs=2)
                            nc.vector.tensor_copy(stage[:], ps_c[0:64, :])
                            bcast = spool.tile([64, 512], f32, tag="bcast",
                                               bufs=2)
                            nc.gpsimd.partition_broadcast(bcast[:], recip[:])
                            cs = slice(b * S + j * 512,
                                       b * S + (j + 1) * 512)
                            nc.vector.tensor_tensor(ctxT_sb[hp, cs],
                                                    stage[:], bcast[:],
                                                    ALU.mult)
                        return go

                    steps.append(mk_norm())
                return steps

            def emit_pe_warm(n):
                # dummy matmuls that keep the PE busy (and at max pstate)
                # while the final collective's peer-wait + transfer drain;
                # results are never read
                for i in range(n):
                    ps_w = psA.tile([128, 1024], f32, tag="psA",
                                    name="ps_warm")
                    nc.tensor.matmul(ps_w[:, 0:512],
                                     lhsT=wqk_sb[:, 0, 0:128],
                                     rhs=qT_sb[:, 0:512],
                                     start=True, stop=True)

            filler = []

            def run_filler(n):
                for _ in range(min(n, len(filler))):
                    filler.pop(0)()

            def drain_filler(before_last=None):
                while filler:
                    if before_last is not None and len(filler) == 1:
                        before_last()
                        before_last = None
                    filler.pop(0)()

            def emit_window(b, j):
                nkt = 4 * (j + 1)
                exp_js = []
                for hl in range(HPC):
                    t = epool.tile([128, nkt * 512], bf16,
                                   tag=f"expj{j}h{hl}", name="exp_j")
                    exp_tiles[(j, hl)] = t
                    exp_js.append(t)
                # Scores pieces: off-diagonal k-tiles (full 512-q) in pairs,
                # then the 4 diagonal tiles (ragged: tile 4j+o covers the
                # last 512-128*o q columns) packed into two PSUM tiles.
                win = b * S + j * 512
                pieces = []
                tt = 0
                while tt < 4 * j:
                    npc = min(2, 4 * j - tt)
                    pieces.append([(tt + i, 0) for i in range(npc)])
                    tt += npc
                pieces.append([(4 * j, 0), (4 * j + 1, 128)])
                pieces.append([(4 * j + 2, 256), (4 * j + 3, 384)])
                n_pieces = len(pieces)
                for pi, piece in enumerate(pieces):
                    ps_h = [psA.tile([128, 1024], f32, tag="psA",
                                     name="ps_sc")
                            for _ in range(HPC)]
                    # pack spans so no matmul output crosses a 512-col
                    # (2KB) PSUM bank boundary
                    col = 0
                    spans = []
                    for (tile_idx, qoff) in piece:
                        w = 512 - qoff
                        if col // 512 != (col + w - 1) // 512:
                            col = ((col + 511) // 512) * 512
                        spans.append((tile_idx, qoff, col, w))
                        col += w
                    for (tile_idx, qoff, c0, w) in spans:
                        kt = b * S + tile_idx * 128
                        for hl in range(HPC):
                            nc.tensor.matmul(
                                ps_h[hl][:, c0:c0 + w],
                                lhsT=kT_sb[:, kt:kt + 128],
                                rhs=qTp[hl][:, win + qoff:win + 512],
                                start=True, stop=True)
                    # exp: one ACT op per head per src/dst-contiguous run
                    for hl in range(HPC):
                        run = []
                        for (tile_idx, qoff, c0, w) in spans:
                            dst = tile_idx * 512 + qoff
                            if run and run[-1][1] + run[-1][2] == dst \
                                    and run[-1][0] + run[-1][2] == c0:
                                run[-1] = (run[-1][0], run[-1][1],
                                           run[-1][2] + w)
                            else:
                                run.append((c0, dst, w))
                        for (c0, dst, w) in run:
                            nc.scalar.activation(
                                exp_js[hl][:, dst:dst + w],
                                ps_h[hl][:, c0:c0 + w], ACTF.Exp)
                    # triangular causal mask on the diagonal tiles' first
                    # 128 valid columns, emitted right after their exp so
                    # they don't queue behind the norm chains on Vector
                    if pi >= n_pieces - 2:
                        for hl in range(HPC):
                            for (tile_idx, qoff, c0, w) in spans:
                                lo = tile_idx * 512 + qoff
                                nc.vector.tensor_tensor(
                                    exp_js[hl][:, lo:lo + 128],
                                    exp_js[hl][:, lo:lo + 128],
                                    mask_sb[:], ALU.mult)
                    # interleave AV filler of the previous window so the
                    # in-order PE has work while Scalar drains exp
                    if filler and (pi % 2 == 1 or pi == n_pieces - 1):
                        rem = (n_pieces - pi + 1) // 2
                        per = -(-len(filler) // max(rem, 1))
                        run_filler(per)
                drain_filler()
                filler.extend(build_av_steps(b, j))

            scope1 = nc.named_scope("qkv"); scope1.__enter__()
            emit_qkv_round(0)
            emit_qkv_round(1)
            emit_v_transposes(0)
            emit_v_transposes(1)
            scope1.__exit__(None, None, None)

            scope2 = nc.named_scope("attn"); scope2.__enter__()
            # batch-0 windows run consecutively so the first two collectives
            # fire as early as possible — the CC queue (4 meshes x ~20-30us,
            # strictly serial) is the tail's critical path. qkv rounds 2/3
            # slot in after batch 0; their x chunks are pre-issued here.
            emit_window(0, 0)
            emit_x_load(2)
            nc.scalar.dma_start(wo_sb[:], wo[:])
            nc.scalar.dma_start(bo_sb[:], bo[:])
            emit_window(0, 1)
            emit_x_load(3)
            emit_window(0, 2)
            emit_a2a(0, 0)             # data (norms of 0,0/0,1) ready
            emit_window(0, 3)
            drain_filler()             # AV(0,3) + norms
            emit_a2a(0, 1)
            emit_qkv_round(2)
            emit_v_transposes(2)
            emit_qkv_round(3)
            emit_v_transposes(3)
            emit_window(1, 0)
            emit_window(1, 1)
            ag0 = load_ctxag(0)
            emit_window(1, 2)
            emit_a2a(1, 0)
            ag1 = load_ctxag(1)
            emit_window(1, 3)
            drain_filler()             # AV(1,3) + norms
            ag2 = load_ctxag(2)
            emit_proj(0, ag0)
            emit_proj(1, ag1)
            emit_proj(2, ag2)
            emit_a2a(1, 1)
            ag3 = load_ctxag(3)
            emit_proj(3, ag3)
            scope2.__exit__(None, None, None)

    nc.compile()
    return nc


def _prep_inputs(x, Wqkv, bqkv, Wo, bo):
    x = np.asarray(x, dtype=np.float32)
    Wqkv = np.asarray(Wqkv, dtype=np.float32)
    bqkv = np.asarray(bqkv, dtype=np.float32)
    Wo = np.asarray(Wo, dtype=np.float32)
    bo = np.asarray(bo, dtype=np.float32)

    xT = x.reshape(BS, D).T.astype(BF16)               # [D, BS]
    # per-round per-partition-contiguous tiles: xr[r][p, ko, t] =
    # xT[ko*128+p, r*1024+t]
    xr = np.ascontiguousarray(
        xT.reshape(KC, 128, NR, 1024).transpose(2, 1, 0, 3))
    wo_t = np.ascontiguousarray(
        Wo.astype(BF16).reshape(KC, 128, D).transpose(1, 0, 2))
    bo_t = np.tile(bo.astype(BF16), (128, 1))

    kp = np.arange(128)[:, None]
    u = np.arange(128)[None, :]
    mask = (u >= kp).astype(BF16)
    ident = np.eye(128, dtype=BF16)

    scale = np.float32(1.0 / np.sqrt(HD))

    # Wqkv columns per head h: q = 192h..+64, k = +64, v = +128
    W3 = Wqkv.reshape(D, H, 3, HD)
    b3 = bqkv.reshape(H, 3, HD)

    in_maps = []
    for c in range(NC):
        hs = [HPC * c + i for i in range(HPC)]
        wq = np.concatenate([W3[:, h, 0, :] for h in hs], axis=1) * scale
        wk = np.concatenate([W3[:, h, 1, :] for h in hs], axis=1)
        wv_ = np.concatenate([W3[:, h, 2, :] for h in hs], axis=1)
        bq = np.concatenate([b3[h, 0, :] for h in hs]) * scale
        bk = np.concatenate([b3[h, 1, :] for h in hs])
        bv_ = np.concatenate([b3[h, 2, :] for h in hs])
        wqk_c = np.concatenate([wq, wk], axis=1).astype(BF16)   # [D, 256]
        wqk_t = np.ascontiguousarray(
            wqk_c.reshape(KC, 128, 256).transpose(1, 0, 2))
        wv_t = np.ascontiguousarray(
            wv_.astype(BF16).reshape(KC, 128, 128).transpose(1, 0, 2))
        m = {
            "wqk": wqk_t,
            "wv": wv_t,
            "wo": wo_t,
            "bqk": np.ascontiguousarray(
                np.stack([bq, bk], axis=1)).astype(np.float32),
            "bv": bv_.astype(np.float32).reshape(128, 1),
            "bo": bo_t,
            "mask": mask,
            "ident": ident,
        }
        for r in range(NR):
            m[f"xr{r}"] = xr[r]
        in_maps.append(m)
    return in_maps


def run(x, Wqkv, bqkv, Wo, bo, trace=False):
    from concourse.bass_utils import run_bass_kernel_spmd

    if "nc" not in _CACHE:
        _CACHE["nc"] = _build_program()
    nc = _CACHE["nc"]
    in_maps = _prep_inputs(x, Wqkv, bqkv, Wo, bo)
    res = run_bass_kernel_spmd(nc, in_maps, list(range(NC)), trace=trace)
    # core c returns [512, D]: 4 chunks of 128 rows: (b0 rows 128c..),
    # (b0 rows 1024+128c..), (b1 rows 128c..), (b1 rows 1024+128c..)
    full = np.empty((B, S, D), dtype=np.float32)
    for c in range(NC):
        r = res.results[c]["out"]
        for g in range(4):
            b, half = g // 2, g % 2
            lo = half * 1024 + 128 * c
            full[b, lo:lo + 128, :] = r[g * 128:(g + 1) * 128, :]
    return full, res


def kernel(x, Wqkv, bqkv, Wo, bo):
    out, _ = run(x, Wqkv, bqkv, Wo, bo)
    return out


# revision 20
# speedup vs baseline: 1.2595x; 1.0375x over previous
"""Trainium2 Bass kernel for causal multi-head attention.

Problem: B=2, S=2048, D=1024, H=16 heads (hd=64), fp32 in/out.
  qkv = x @ Wqkv + bqkv ; per-head causal softmax attention ; out = ctx @ Wo + bo

Sharding (8 NeuronCores): tensor-parallel over heads — 2 heads per core.
Each core computes q/k/v projections for its 2 heads (both batches), causal
attention, and its ctx^T slice [128 feat, B*S]. Four AllToAll collectives
(one per (batch, half), 256KB each) redistribute ctx^T from head-sharded to
row-sharded; each core then projects 128 rows per chunk with the full Wo.
Host reassembles the row slices.

v4 vs v2:
- All big inputs (x rounds, wqk, wv, wo) pre-tiled host-side into
  per-partition-contiguous DRAM layouts: DMA issue drops from ~1-8us to
  ~0.2us each and transfers run at full HBM rate; first matmul starts ~6us
  earlier.
- Ragged AV: the attn@v accumulation skips the causally-invalid q-ranges of
  the diagonal k-tiles (N=512-128*o) instead of multiplying zeroed exp.
  Saves ~25% of AV matmul columns and drops the exp-tile zero memsets.
- Fine-grained PE interleave: the AV matmuls of window j-1 are emitted
  between the scores pieces of window j (the PE is in-order, so filler must
  be interleaved in emission order). The PE no longer stalls while the
  Scalar engine drains exp.
- The per-batch AllToAll is split into four per-(batch, half) collectives,
  each fired as soon as its two windows' ctx is normalized; projections are
  spread between late windows so only a 256KB collective + one projection
  remain in the tail.

Numerics: bf16 matmul operands, fp32 PSUM accumulation. Softmax uses
exp without max-subtraction (scores are ~N(0,1) after the folded 1/sqrt(hd)
scale; |s| < ~8 so fp32 exp/sums are safe). The softmax denominator comes
for free as a ones-column appended to v in the attn@v matmul.
"""

import numpy as np
import ml_dtypes

B, S, D, H, NC = 2, 2048, 1024, 16, 8
HD = D // H            # 64
HPC = H // NC          # 2 heads per core
BS = B * S             # 4096
RPB = S // NC          # 256 output rows per core per batch
KC = D // 128          # 8 contraction chunks
NR = 4                 # qkv rounds of 1024 tokens
NKT = S // 128         # 16 k-tiles (128) per batch

BF16 = ml_dtypes.bfloat16

_CACHE = {}


def _build_program():
    import concourse.bass as bass
    import concourse.mybir as mybir
    from concourse import bacc
    from concourse.tile import TileContext

    dt = mybir.dt
    f32, bf16 = dt.float32, dt.bfloat16
    ALU = mybir.AluOpType
    ACTF = mybir.ActivationFunctionType

    nc = bacc.Bacc("TRN2", target_bir_lowering=False, debug=False, num_devices=NC)

    xr_d = [nc.dram_tensor(f"xr{r}", [128, KC, 1024], bf16,
                           kind="ExternalInput") for r in range(NR)]
    wqk = nc.dram_tensor("wqk", [128, KC, 256], bf16, kind="ExternalInput")
    wv = nc.dram_tensor("wv", [128, KC, 128], bf16, kind="ExternalInput")
    wo = nc.dram_tensor("wo", [128, KC, D], bf16, kind="ExternalInput")
    bqk = nc.dram_tensor("bqk", [128, 2], f32, kind="ExternalInput")
    bv = nc.dram_tensor("bv", [128, 1], f32, kind="ExternalInput")
    bo = nc.dram_tensor("bo", [128, D], bf16, kind="ExternalInput")
    mask = nc.dram_tensor("mask", [128, 128], bf16, kind="ExternalInput")
    ident = nc.dram_tensor("ident", [128, 128], bf16, kind="ExternalInput")
    out = nc.dram_tensor("out", [2 * RPB, D], f32, kind="ExternalOutput")

    # collective buffers: one AllToAll per (batch, half). Shard j of the
    # send buffer = [our 128 feats, core j's 128 q rows of this half];
    # the received shard j = [core j's 128 feats, our 128 rows].
    ctx_dram = [[nc.dram_tensor(f"ctxb{g}h{h}", [NC, 128, 128], bf16)
                 for h in range(2)] for g in range(B)]
    a2a_dram = [[nc.dram_tensor(f"ctxa2a{g}h{h}", [NC, 128, 128], bf16)
                 for h in range(2)] for g in range(B)]

    with TileContext(nc) as tc:
        with (
            tc.tile_pool(name="const", bufs=1) as cpool,
            tc.tile_pool(name="big", bufs=1) as bigpool,
            tc.tile_pool(name="xstream", bufs=2) as xpool,
            tc.tile_pool(name="vt", bufs=2) as vtpool,
            tc.tile_pool(name="exp", bufs=1) as epool,
            tc.tile_pool(name="small", bufs=2) as spool,
            tc.tile_pool(name="ag", bufs=3) as agpool,
            tc.tile_pool(name="outp", bufs=2) as opool,
            tc.tile_pool(name="psA", bufs=2, space="PSUM") as psA,   # 2x [128,1024]
            tc.tile_pool(name="psB", bufs=4, space="PSUM") as psB,   # 4x [128,512]
        ):
            # ---- constants / weights to SBUF ----
            # wqk + round-0 x gate the whole kernel: queue them first, in
            # arrival-need order, all per-partition-contiguous in DRAM.
            wqk_sb = cpool.tile([128, KC, 256], bf16, tag="wqk")
            x0 = xpool.tile([128, KC, 1024], bf16, tag="xt", name="x0")
            x1 = xpool.tile([128, KC, 1024], bf16, tag="xt", name="x1")
            # per-queue DMA bandwidth saturates well below HBM rate, so the
            # x stream is split across the sync and scalar queues
            # fine-grained startup chunks round-robined over the three
            # DMA-capable queues: the first matmul needs only wqk kk0-1 +
            # x0 kk0, and the aggregate stream keeps round 0/1 fed densely
            qs = [nc.sync, nc.scalar, nc.gpsimd]
            nc.sync.dma_start(wqk_sb[:, 0:1, :], wqk[:, 0:1, :])
            nc.scalar.dma_start(x0[:, 0:1, 0:512], xr_d[0][:, 0:1, 0:512])
            nc.gpsimd.dma_start(x0[:, 0:1, 512:1024],
                                xr_d[0][:, 0:1, 512:1024])
            nc.sync.dma_start(wqk_sb[:, 1:2, :], wqk[:, 1:2, :])
            nc.scalar.dma_start(wqk_sb[:, 2:4, :], wqk[:, 2:4, :])
            nc.gpsimd.dma_start(wqk_sb[:, 4:6, :], wqk[:, 4:6, :])
            nc.sync.dma_start(wqk_sb[:, 6:8, :], wqk[:, 6:8, :])
            for kk in range(1, KC):
                qs[kk % 3].dma_start(x0[:, kk:kk + 1, :],
                                     xr_d[0][:, kk:kk + 1, :])
            for c in range(4):
                qs[c % 3].dma_start(x1[:, 2 * c:2 * c + 2, :],
                                    xr_d[1][:, 2 * c:2 * c + 2, :])
            wv_sb = cpool.tile([128, KC, 128], bf16, tag="wv")
            nc.scalar.dma_start(wv_sb[:], wv[:])
            bqk_sb = cpool.tile([128, 2], f32, tag="bqk")
            nc.scalar.dma_start(bqk_sb[:], bqk[:])
            bv_sb = cpool.tile([128, 1], f32, tag="bv")
            nc.scalar.dma_start(bv_sb[:], bv[:])
            mask_sb = cpool.tile([128, 128], bf16, tag="mask")
            nc.scalar.dma_start(mask_sb[:], mask[:])
            ident_sb = cpool.tile([128, 128], bf16, tag="ident")
            nc.scalar.dma_start(ident_sb[:], ident[:])
            wo_sb = cpool.tile([128, KC, D], bf16, tag="wo")
            bo_sb = cpool.tile([128, D], bf16, tag="bo")

            # ---- persistent activations ----
            # q stored twice, one copy per head with the other head's rows
            # zeroed: scores matmuls contract K=128 so the PE stays in
            # 128-row tile mode (64-row mode entries drain the array)
            qTp = [bigpool.tile([128, BS], bf16, tag=f"qT{hl}",
                                name=f"qT{hl}")
                   for hl in range(HPC)]
            kT_sb = bigpool.tile([128, BS], bf16, tag="kT")
            # v natural layout: [kpos, tile, head, 65] with ones at col 64
            v_sb = bigpool.tile([128, BS // 128, HPC, 65], bf16, tag="v")
            ctxT_sb = bigpool.tile([128, BS], bf16, tag="ctxT")

            nc.vector.memset(v_sb[:, :, :, 64:65], 1.0)
            nc.vector.memset(qTp[0][64:128, :], 0.0)
            nc.vector.memset(qTp[1][0:64, :], 0.0)

            vt_tiles = {}
            exp_tiles = {}

            # ---- qkv projection: rounds of 1024 tokens ----
            x_tiles = {0: x0, 1: x1}

            def emit_x_load(r):
                xt = xpool.tile([128, KC, 1024], bf16, tag="xt", name="xt")
                x_tiles[r] = xt
                eng2 = nc.scalar if r == 2 else nc.gpsimd
                for c in range(4):
                    eng = nc.sync if c % 2 == 0 else eng2
                    eng.dma_start(xt[:, 2 * c:2 * c + 2, :],
                                  xr_d[r][:, 2 * c:2 * c + 2, :])

            def emit_qkv_round(r):
                lo = r * 1024
                xt = x_tiles[r]

                ps_q = psA.tile([128, 1024], f32, tag="psA", name="ps_q")
                ps_k = psA.tile([128, 1024], f32, tag="psA", name="ps_k")
                ps_v0 = psB.tile([128, 512], f32, tag="psB", name="ps_v0")
                ps_v1 = psB.tile([128, 512], f32, tag="psB", name="ps_v1")
                for kk in range(KC):
                    nc.tensor.matmul(ps_q[:, 0:512], lhsT=wqk_sb[:, kk, 0:128],
                                     rhs=xt[:, kk, 0:512],
                                     start=(kk == 0), stop=(kk == KC - 1))
                    nc.tensor.matmul(ps_q[:, 512:1024], lhsT=wqk_sb[:, kk, 0:128],
                                     rhs=xt[:, kk, 512:1024],
                                     start=(kk == 0), stop=(kk == KC - 1))
                for kk in range(KC):
                    nc.tensor.matmul(ps_k[:, 0:512], lhsT=wqk_sb[:, kk, 128:256],
                                     rhs=xt[:, kk, 0:512],
                                     start=(kk == 0), stop=(kk == KC - 1))
                    nc.tensor.matmul(ps_k[:, 512:1024], lhsT=wqk_sb[:, kk, 128:256],
                                     rhs=xt[:, kk, 512:1024],
                                     start=(kk == 0), stop=(kk == KC - 1))
                for kk in range(KC):
                    nc.tensor.matmul(ps_v0, lhsT=wv_sb[:, kk, :],
                                     rhs=xt[:, kk, 0:512],
                                     start=(kk == 0), stop=(kk == KC - 1))
                    nc.tensor.matmul(ps_v1, lhsT=wv_sb[:, kk, :],
                                     rhs=xt[:, kk, 512:1024],
                                     start=(kk == 0), stop=(kk == KC - 1))
                vt = vtpool.tile([128, 1024], bf16, tag="vT", name="vt")
                vt_tiles[r] = vt
                nc.vector.tensor_scalar_add(qTp[0][0:64, lo:lo + 1024],
                                            ps_q[0:64, :], bqk_sb[0:64, 0:1])
                nc.vector.tensor_scalar_add(qTp[1][64:128, lo:lo + 1024],
                                            ps_q[64:128, :],
                                            bqk_sb[64:128, 0:1])
                nc.vector.tensor_scalar_add(kT_sb[:, lo:lo + 1024], ps_k,
                                            bqk_sb[:, 1:2])
                nc.vector.tensor_scalar_add(vt[:, 0:512], ps_v0,
                                            bv_sb[:, 0:1])
                nc.vector.tensor_scalar_add(vt[:, 512:1024], ps_v1,
                                            bv_sb[:, 0:1])

            # v^T [feat, tok] -> natural [tok, feat] via PE transposes,
            # 8 tiles packed per PSUM tile, drained by one strided DVE copy.
            def emit_v_transposes(r):
                vt = vt_tiles.pop(r)
                pack = psA.tile([128, 8, HPC, 64], bf16, tag="psA", name="tpack")
                for t8 in range(8):
                    c0 = t8 * 128
                    nc.tensor.transpose(pack[:, t8], vt[:, c0:c0 + 128],
                                        ident_sb[:])
                nc.vector.tensor_copy(v_sb[:, r * 8:(r + 1) * 8, :, 0:64],
                                      pack[:])

            # ---- collectives + output projection ----
            def emit_a2a(b, h):
                # store split per source window: the first window's half was
                # normalized a window earlier and transfers immediately,
                # leaving only 128KB on the trigger's critical path
                dst = ctx_dram[b][h].rearrange("j p s -> p j s")
                c0 = b * S + h * 1024
                nc.gpsimd.dma_start(dst[:, 0:4, :], ctxT_sb[:, c0:c0 + 512])
                nc.gpsimd.dma_start(dst[:, 4:8, :],
                                    ctxT_sb[:, c0 + 512:c0 + 1024])
                nc.gpsimd.collective_compute(
                    "AllToAll",
                    mybir.AluOpType.bypass,
                    replica_groups=[list(range(NC))],
                    ins=[ctx_dram[b][h][:]],
                    outs=[a2a_dram[b][h][:]],
                )

            def load_ctxag(g):
                b, half = g // 2, g % 2
                ctxag_sb = agpool.tile([128, NC, 128], bf16, tag="ctxag",
                                       name="ctxag_sb")
                src_v = a2a_dram[b][half].rearrange("j p s -> p j s")
                nc.sync.dma_start(ctxag_sb[:, 0:4, :], src_v[:, 0:4, :])
                nc.sync.dma_start(ctxag_sb[:, 4:8, :], src_v[:, 4:8, :])
                return ctxag_sb

            def emit_proj(g, ctxag_sb=None):
                if ctxag_sb is None:
                    ctxag_sb = load_ctxag(g)
                ps_o = psA.tile([128, 1024], f32, tag="psA", name="ps_o")
                # sequential halves: half-0's bias-add + store overlap
                # half-1's matmuls, shortening the tail after the last a2a
                for half2 in range(2):
                    cs = slice(512 * half2, 512 * half2 + 512)
                    for k in range(NC):
                        nc.tensor.matmul(ps_o[:, cs],
                                         lhsT=ctxag_sb[:, k, :],
                                         rhs=wo_sb[:, k, cs],
                                         start=(k == 0), stop=(k == NC - 1))
                    ot = opool.tile([128, 512], f32, tag="ot")
                    nc.vector.tensor_tensor(ot[:], ps_o[:, cs],
                                            bo_sb[:, cs], ALU.add)
                    nc.sync.dma_start(out[g * 128:(g + 1) * 128, cs], ot[:])

            # ---- AV + normalize, emitted as fine-grained filler ----
            # Returns a list of closures: PE matmul chunks (interleaved
            # between the next window's scores pieces to keep the in-order
            # PE busy) followed by one normalize closure per head (vector/
            # gpsimd work, order-free).
            def build_av_steps(b, j):
                steps = []
                nkt = 4 * (j + 1)
                for hl in range(HPC):
                    exp_j = exp_tiles[(j, hl)]
                    ps_c = psB.tile([128, 512], f32, tag="psB", name="ps_c")
                    # full k-tiles at N=512, then ragged diagonal tiles at
                    # N=512-128*o (the causally-invalid prefix is skipped)
                    mms = [(tt, 0) for tt in range(4 * j)]
                    mms += [(4 * j + o, 128 * o) for o in range(4)]

                    def mk_mm(pair, first, last, hl=hl, exp_j=exp_j,
                              ps_c=ps_c):
                        def go():
                            for i, (tt, q0) in enumerate(pair):
                                nc.tensor.matmul(
                                    ps_c[:65, q0:512],
                                    lhsT=v_sb[:, b * NKT + tt, hl, :],
                                    rhs=exp_j[:, tt * 512 + q0:
                                              (tt + 1) * 512],
                                    start=(first and i == 0),
                                    stop=(last and i == len(pair) - 1))
                        return go

                    for ci in range(0, len(mms), 2):
                        pair = mms[ci:ci + 2]
                        steps.append(mk_mm(pair, ci == 0,
                                           ci + 2 >= len(mms)))

                    def mk_norm(hl=hl, ps_c=ps_c):
                        def go():
                            hp = slice(64 * hl, 64 * hl + 64)
                            # den/recip read PSUM directly, in parallel with
                            # the stage copy — shortens the serial chain to
                            # the a2a trigger by ~3us
                            den = spool.tile([1, 512], f32, tag="den")
                            nc.vector.tensor_copy(den[:], ps_c[64:65, :])
                            recip = spool.tile([1, 512], f32, tag="recip")
                            nc.vector.reciprocal_approx_fast(out=recip[:],
                                                             in_=den[:])
                            stage = spool.tile([64, 512], f32, tag="stage",
                                               bufs=2)
                            nc.vector.tensor_copy(stage[:], ps_c[0:64, :])
                            bcast = spool.tile([64, 512], f32, tag="bcast",
                                               bufs=2)
                            nc.gpsimd.partition_broadcast(bcast[:], recip[:])
                            cs = slice(b * S + j * 512,
                                       b * S + (j + 1) * 512)
                            nc.vector.tensor_tensor(ctxT_sb[hp, cs],
                                                    stage[:], bcast[:],
                                                    ALU.mult)
                        return go

                    steps.append(mk_norm())
                return steps

            def emit_pe_warm(n):
                # dummy matmuls that keep the PE busy (and at max pstate)
                # while the final collective's peer-wait + transfer drain;
                # results are never read
                for i in range(n):
                    ps_w = psA.tile([128, 1024], f32, tag="psA",
                                    name="ps_warm")
                    nc.tensor.matmul(ps_w[:, 0:512],
                                     lhsT=wqk_sb[:, 0, 0:128],
                                     rhs=qT_sb[:, 0:512],
                                     start=True, stop=True)

            filler = []

            def run_filler(n):
                for _ in range(min(n, len(filler))):
                    filler.pop(0)()

            def drain_filler(before_last=None):
                while filler:
                    if before_last is not None and len(filler) == 1:
                        before_last()
                        before_last = None
                    filler.pop(0)()

            def emit_window(b, j):
                nkt = 4 * (j + 1)
                exp_js = []
                for hl in range(HPC):
                    t = epool.tile([128, nkt * 512], bf16,
                                   tag=f"expj{j}h{hl}", name="exp_j")
                    exp_tiles[(j, hl)] = t
                    exp_js.append(t)
                # Scores pieces: off-diagonal k-tiles (full 512-q) in pairs,
                # then the 4 diagonal tiles (ragged: tile 4j+o covers the
                # last 512-128*o q columns) packed into two PSUM tiles.
                win = b * S + j * 512
                pieces = []
                tt = 0
                while tt < 4 * j:
                    npc = min(2, 4 * j - tt)
                    pieces.append([(tt + i, 0) for i in range(npc)])
                    tt += npc
                pieces.append([(4 * j, 0), (4 * j + 1, 128)])
                pieces.append([(4 * j + 2, 256), (4 * j + 3, 384)])
                n_pieces = len(pieces)
                for pi, piece in enumerate(pieces):
                    ps_h = [psA.tile([128, 1024], f32, tag="psA",
                                     name="ps_sc")
                            for _ in range(HPC)]
                    # pack spans so no matmul output crosses a 512-col
                    # (2KB) PSUM bank boundary
                    col = 0
                    spans = []
                    for (tile_idx, qoff) in piece:
                        w = 512 - qoff
                        if col // 512 != (col + w - 1) // 512:
                            col = ((col + 511) // 512) * 512
                        spans.append((tile_idx, qoff, col, w))
                        col += w
                    for (tile_idx, qoff, c0, w) in spans:
                        kt = b * S + tile_idx * 128
                        for hl in range(HPC):
                            nc.tensor.matmul(
                                ps_h[hl][:, c0:c0 + w],
                                lhsT=kT_sb[:, kt:kt + 128],
                                rhs=qTp[hl][:, win + qoff:win + 512],
                                start=True, stop=True)
                    # exp: one ACT op per head per src/dst-contiguous run
                    for hl in range(HPC):
                        run = []
                        for (tile_idx, qoff, c0, w) in spans:
                            dst = tile_idx * 512 + qoff
                            if run and run[-1][1] + run[-1][2] == dst \
                                    and run[-1][0] + run[-1][2] == c0:
                                run[-1] = (run[-1][0], run[-1][1],
                                           run[-1][2] + w)
                            else:
                                run.append((c0, dst, w))
                        for (c0, dst, w) in run:
                            nc.scalar.activation(
                                exp_js[hl][:, dst:dst + w],
                                ps_h[hl][:, c0:c0 + w], ACTF.Exp)
                    # triangular causal mask on the diagonal tiles' first
                    # 128 valid columns, emitted right after their exp so
                    # they don't queue behind the norm chains on Vector
                    if pi >= n_pieces - 2:
                        for hl in range(HPC):
                            for (tile_idx, qoff, c0, w) in spans:
                                lo = tile_idx * 512 + qoff
                                nc.vector.tensor_tensor(
                                    exp_js[hl][:, lo:lo + 128],
                                    exp_js[hl][:, lo:lo + 128],
                                    mask_sb[:], ALU.mult)
                    # interleave AV filler of the previous window so the
                    # in-order PE has work while Scalar drains exp
                    if filler and (pi % 2 == 1 or pi == n_pieces - 1):
                        rem = (n_pieces - pi + 1) // 2
                        per = -(-len(filler) // max(rem, 1))
                        run_filler(per)
                drain_filler()
                filler.extend(build_av_steps(b, j))

            scope1 = nc.named_scope("qkv"); scope1.__enter__()
            emit_qkv_round(0)
            emit_qkv_round(1)
            emit_v_transposes(0)
            emit_v_transposes(1)
            scope1.__exit__(None, None, None)

            scope2 = nc.named_scope("attn"); scope2.__enter__()
            # batch-0 windows run consecutively so the first two collectives
            # fire as early as possible — the CC queue (4 meshes x ~20-30us,
            # strictly serial) is the tail's critical path. qkv rounds 2/3
            # slot in after batch 0; their x chunks are pre-issued here.
            emit_window(0, 0)
            emit_x_load(2)
            nc.scalar.dma_start(wo_sb[:], wo[:])
            nc.scalar.dma_start(bo_sb[:], bo[:])
            emit_window(0, 1)
            emit_x_load(3)
            emit_window(0, 2)
            emit_a2a(0, 0)             # data (norms of 0,0/0,1) ready
            emit_window(0, 3)
            drain_filler()             # AV(0,3) + norms
            emit_a2a(0, 1)
            emit_qkv_round(2)
            emit_v_transposes(2)
            emit_qkv_round(3)
            emit_v_transposes(3)
            emit_window(1, 0)
            emit_window(1, 1)
            ag0 = load_ctxag(0)
            emit_window(1, 2)
            emit_a2a(1, 0)
            ag1 = load_ctxag(1)
            emit_window(1, 3)
            drain_filler()             # AV(1,3) + norms
            ag2 = load_ctxag(2)
            emit_proj(0, ag0)
            emit_proj(1, ag1)
            emit_proj(2, ag2)
            emit_a2a(1, 1)
            ag3 = load_ctxag(3)
            emit_proj(3, ag3)
            scope2.__exit__(None, None, None)

    nc.compile()
    return nc


def _prep_inputs(x, Wqkv, bqkv, Wo, bo):
    x = np.asarray(x, dtype=np.float32)
    Wqkv = np.asarray(Wqkv, dtype=np.float32)
    bqkv = np.asarray(bqkv, dtype=np.float32)
    Wo = np.asarray(Wo, dtype=np.float32)
    bo = np.asarray(bo, dtype=np.float32)

    xT = x.reshape(BS, D).T.astype(BF16)               # [D, BS]
    # per-round per-partition-contiguous tiles: xr[r][p, ko, t] =
    # xT[ko*128+p, r*1024+t]
    xr = np.ascontiguousarray(
        xT.reshape(KC, 128, NR, 1024).transpose(2, 1, 0, 3))
    wo_t = np.ascontiguousarray(
        Wo.astype(BF16).reshape(KC, 128, D).transpose(1, 0, 2))
    bo_t = np.tile(bo.astype(BF16), (128, 1))

    kp = np.arange(128)[:, None]
    u = np.arange(128)[None, :]
    mask = (u >= kp).astype(BF16)
    ident = np.eye(128, dtype=BF16)

    scale = np.float32(1.0 / np.sqrt(HD))

    # Wqkv columns per head h: q = 192h..+64, k = +64, v = +128
    W3 = Wqkv.reshape(D, H, 3, HD)
    b3 = bqkv.reshape(H, 3, HD)

    in_maps = []
    for c in range(NC):
        hs = [HPC * c + i for i in range(HPC)]
        wq = np.concatenate([W3[:, h, 0, :] for h in hs], axis=1) * scale
        wk = np.concatenate([W3[:, h, 1, :] for h in hs], axis=1)
        wv_ = np.concatenate([W3[:, h, 2, :] for h in hs], axis=1)
        bq = np.concatenate([b3[h, 0, :] for h in hs]) * scale
        bk = np.concatenate([b3[h, 1, :] for h in hs])
        bv_ = np.concatenate([b3[h, 2, :] for h in hs])
        wqk_c = np.concatenate([wq, wk], axis=1).astype(BF16)   # [D, 256]
        wqk_t = np.ascontiguousarray(
            wqk_c.reshape(KC, 128, 256).transpose(1, 0, 2))
        wv_t = np.ascontiguousarray(
            wv_.astype(BF16).reshape(KC, 128, 128).transpose(1, 0, 2))
        m = {
            "wqk": wqk_t,
            "wv": wv_t,
            "wo": wo_t,
            "bqk": np.ascontiguousarray(
                np.stack([bq, bk], axis=1)).astype(np.float32),
            "bv": bv_.astype(np.float32).reshape(128, 1),
            "bo": bo_t,
            "mask": mask,
            "ident": ident,
        }
        for r in range(NR):
            m[f"xr{r}"] = xr[r]
        in_maps.append(m)
    return in_maps


def run(x, Wqkv, bqkv, Wo, bo, trace=False):
    from concourse.bass_utils import run_bass_kernel_spmd

    if "nc" not in _CACHE:
        _CACHE["nc"] = _build_program()
    nc = _CACHE["nc"]
    in_maps = _prep_inputs(x, Wqkv, bqkv, Wo, bo)
    res = run_bass_kernel_spmd(nc, in_maps, list(range(NC)), trace=trace)
    # core c returns [512, D]: 4 chunks of 128 rows: (b0 rows 128c..),
    # (b0 rows 1024+128c..), (b1 rows 128c..), (b1 rows 1024+128c..)
    full = np.empty((B, S, D), dtype=np.float32)
    for c in range(NC):
        r = res.results[c]["out"]
        for g in range(4):
            b, half = g // 2, g % 2
            lo = half * 1024 + 128 * c
            full[b, lo:lo + 128, :] = r[g * 128:(g + 1) * 128, :]
    return full, res


def kernel(x, Wqkv, bqkv, Wo, bo):
    out, _ = run(x, Wqkv, bqkv, Wo, bo)
    return out
